# revision 1
# baseline (speedup 1.0000x reference)
"""GAT (2-layer, 4-head) + MLP/BatchNorm predictor on 8 Trainium2 NeuronCores.

Strategy (graph-parallel, dst-sharded):
  - Nodes split contiguously: core c owns dsts [c*6250, (c+1)*6250). Edges live
    with their dst core, sorted by dst, grouped into 49 chunks of <=128 dsts.
  - Per layer, every core computes the FULL projection table (redundant
    compute, zero communication): row n = [h~(n) = (x@W)*al (256 fp16) |
    el(n) 4xf32 | er(n) 4xf32 | pad] = 768B, stored in its own HBM.
  - Edge phase per chunk: dma_gather of src rows (two gathers: table halves,
    since gather indices are int16), attention computed in f32 from packed
    el + er (er broadcast chunk->edges via a mask-transpose matmul), edge
    softmax denominators and weighted feature sums via mask matmuls on the
    TensorEngine accumulating in PSUM; softmax division deferred to node
    space (exact algebra).
  - Collectives: AllGather of the layer-1 output (x2, fp16) so each core can
    build the full layer-2 table; AllReduce of BatchNorm statistics.
"""
import sys

sys.path.insert(0, "/opt/trn_rl_repo")

import numpy as np

N = 50000
F_IN = 128
H = 4
D = 64
HD = 256
NCORES = 8
NSHARD = N // NCORES          # 6250
P = 128
NCHUNK = (NSHARD + P - 1) // P  # 49 (last chunk 106 dsts)
SPLIT = 32768                 # int16 gather index limit


def configure(n, split=32768):
    """Override problem size (for simulator debugging)."""
    global N, NSHARD, NCHUNK, SPLIT
    N = n
    NSHARD = N // NCORES
    NCHUNK = (NSHARD + P - 1) // P
    SPLIT = split
MLP_H = 200
NCLS = 2
NEG = 0.2
EPS = 1e-5
ROW = 384                     # fp16 elems per table row (768 B)
ELOFF = 256                   # fp16-slot offset of f32 el (4 vals = slots 256:264)
EROFF = 264                   # fp16-slot offset of f32 er


# ----------------------------------------------------------------------------
# Host-side preprocessing
# ----------------------------------------------------------------------------

def _fold_weights(W, al, ar):
    """W:[F,H*D] al,ar:[H,D] -> (Wext [F, H*D+8] f16, recip_al [H*D] f32)."""
    F = W.shape[0]
    alf = al.reshape(-1).astype(np.float64)
    Ws = W.astype(np.float64) * alf[None, :]
    Wel = (W.reshape(F, H, D).astype(np.float64) * al[None]).sum(-1)
    Wer = (W.reshape(F, H, D).astype(np.float64) * ar[None]).sum(-1)
    Wext = np.concatenate([Ws, Wel, Wer], axis=1).astype(np.float16)
    recip = (1.0 / alf).astype(np.float32)
    return Wext, recip


def _prep_edges(src, dst):
    """Build per-core gather/mask arrays. Returns (plan, per_core_arrays).

    plan: dict with T_lo[j], T_hi[j] (identical across cores).
    per-core arrays: IDX [128, 8*totT] i16, DLC [128, totT] f32,
                     DLR [1, totT*128] f16.
    """
    src = np.asarray(src)
    dst = np.asarray(dst)
    per_core = []
    for c in range(NCORES):
        m = (dst >= c * NSHARD) & (dst < (c + 1) * NSHARD)
        es, ed = src[m], dst[m] - c * NSHARD
        order = np.argsort(ed, kind="stable")
        es, ed = es[order], ed[order]
        # chunk boundaries
        starts = np.searchsorted(ed, np.arange(0, NCHUNK * P, P))
        ends = np.searchsorted(ed, np.minimum(np.arange(P, (NCHUNK + 1) * P, P), NSHARD))
        chunks = []
        for j in range(NCHUNK):
            cs, ce = starts[j], ends[j]
            s_j, d_j = es[cs:ce], ed[cs:ce] - j * P
            lo = s_j < SPLIT
            chunks.append((s_j[lo], d_j[lo], s_j[~lo] - SPLIT, d_j[~lo]))
        per_core.append(chunks)

    T_lo = np.zeros(NCHUNK, np.int64)
    T_hi = np.zeros(NCHUNK, np.int64)
    for c in range(NCORES):
        for j in range(NCHUNK):
            slo, _, shi, _ = per_core[c][j]
            T_lo[j] = max(T_lo[j], -(-len(slo) // P))
            T_hi[j] = max(T_hi[j], -(-len(shi) // P))
    T_lo = np.maximum(T_lo, 1)  # every chunk has self-loops -> lo nonzero anyway
    totT = int((T_lo + T_hi).sum())

    def wrap_idx(flat):
        """dma_gather index layout: idx j at [16*rep + j%16, j//16], rep 0..7."""
        n = len(flat)
        cols = n // 16
        a = flat.reshape(cols, 16).T.astype(np.int16)      # [16, cols]
        return np.tile(a, (8, 1))                          # [128, cols]

    arrays = []
    for c in range(NCORES):
        idx_cols = []
        dlc = np.zeros((P, totT), np.float32)
        dlr = np.zeros(totT * P, np.float16)
        t0 = 0
        for j in range(NCHUNK):
            slo, dlo, shi, dhi = per_core[c][j]
            for (s_j, d_j, T) in ((slo, dlo, T_lo[j]), (shi, dhi, T_hi[j])):
                nslot = int(T) * P
                if nslot == 0:
                    continue
                idx = np.zeros(nslot, np.int16)
                dl = np.full(nslot, -1.0, np.float32)
                idx[: len(s_j)] = s_j
                dl[: len(s_j)] = d_j
                idx_cols.append(wrap_idx(idx))
                # slot s = t*128+p  ->  dlc[p, t0+t], dlr[(t0+t)*128 + p]
                dlm = dl.reshape(int(T), P)                 # [t, p]
                dlc[:, t0 : t0 + int(T)] = dlm.T
                dlr[t0 * P : (t0 + int(T)) * P] = dl.astype(np.float16)
                t0 += int(T)
        assert t0 == totT
        IDX = np.concatenate(idx_cols, axis=1)
        assert IDX.shape == (P, 8 * totT)
        arrays.append((IDX, dlc, dlr.reshape(1, totT * P)))

    plan = {"T_lo": T_lo.tolist(), "T_hi": T_hi.tolist(), "totT": totT}
    return plan, arrays


# ----------------------------------------------------------------------------
# Bass program
# ----------------------------------------------------------------------------

def build_nc(plan, phases='full', taps=False, max_chunks=None, ev=3, reps=1):
    import concourse.bacc as bacc
    import concourse.bass as bass
    import concourse.tile as tile
    from concourse import mybir

    FP16 = mybir.dt.float16
    F32 = mybir.dt.float32
    I16 = mybir.dt.int16
    ALU = mybir.AluOpType
    ACTF = mybir.ActivationFunctionType

    T_lo, T_hi, totT = plan["T_lo"], plan["T_hi"], plan["totT"]
    NTILE = (N + P - 1) // P            # 391 (last 80 rows)

    nc = bacc.Bacc("TRN2", target_bir_lowering=False, debug=False,
                   num_devices=NCORES)

    dp = lambda name, shape, dt: nc.declare_dram_parameter(name, shape, dt, isOutput=False)
    features = dp("features", [N, F_IN], FP16)
    fown = dp("fown", [NSHARD, F_IN], FP16)
    IDX = dp("IDX", [P, 8 * totT], I16)
    DLC = dp("DLC", [P, totT], F32)
    DLR = dp("DLR", [1, totT * P], FP16)
    IDENT = dp("IDENT", [P, P], FP16)
    IOTAROW = dp("IOTAROW", [P, P], FP16)
    IOTACOL = dp("IOTACOL", [P, 1], F32)
    W1EXT = dp("W1EXT", [F_IN, HD + 8], FP16)
    W2EXT = dp("W2EXT", [D, HD + 8], FP16)
    RECIP1 = dp("RECIP1", [P, HD], F32)
    RECIP2 = dp("RECIP2", [P, HD], F32)
    B1 = dp("B1", [P, HD], F32)
    B2 = dp("B2", [P, HD], F32)
    WM1 = dp("WM1", [D, MLP_H], FP16)
    BM1 = dp("BM1", [P, MLP_H], F32)
    WM2C1 = dp("WM2C1", [P, NCLS], FP16)
    WM2C2 = dp("WM2C2", [MLP_H - P, NCLS], FP16)
    GB = dp("GB", [P, 4], F32)          # cols: gamma_c1, gamma_c2, beta_c1, beta_c2
    BM2 = dp("BM2", [1, NCLS], F32)
    ONESC = dp("ONESC", [P, 1], FP16)
    IDENTF = dp("IDENTF", [P, P], F32)
    ONESF = dp("ONESF", [P, 1], F32)
    WM1F = dp("WM1F", [D, MLP_H], F32)
    WM2C1F = dp("WM2C1F", [P, NCLS], F32)
    WM2C2F = dp("WM2C2F", [MLP_H - P, NCLS], F32)

    out = nc.declare_dram_parameter("out", [NSHARD, NCLS], F32, isOutput=True)

    table1 = nc.dram_tensor("table1", [N, ROW], FP16)
    table2 = nc.dram_tensor("table2", [N, ROW], FP16)
    x2slice = nc.dram_tensor("x2slice", [NSHARD, D], FP16)
    x3slice = nc.dram_tensor("x3slice", [NSHARD, D], F32)
    x2full = nc.dram_tensor("x2full", [N, D], FP16, addr_space="Shared")
    ccin = nc.dram_tensor("ccin", [P, 4], F32)
    ccout = nc.dram_tensor("ccout", [P, 4], F32, addr_space="Shared")
    ccin1 = nc.dram_tensor("ccin1", [P, 2], F32)
    ccout1 = nc.dram_tensor("ccout1", [P, 2], F32, addr_space="Shared")
    muraw = nc.dram_tensor("muraw", [1, MLP_H], F32)

    def bcast_ap(ap_row):
        """[1, n] DRAM AP -> [[0,128], ...] partition-broadcast AP."""
        return bass.AP(tensor=ap_row.tensor, offset=ap_row.offset,
                       ap=[[0, P]] + ap_row.ap[1:])

    class _SkipRest(Exception):
        pass

    with tile.TileContext(nc) as tc:
        import contextlib
        try:
          with contextlib.ExitStack() as ctx:
            singles = ctx.enter_context(tc.tile_pool(name="singles", bufs=1))

            def load_const(param, shape, dtype, tag):
                t = singles.tile(shape, dtype, tag=tag)
                nc.sync.dma_start(out=t[:], in_=param[:])
                return t

            ident = load_const(IDENT, [P, P], FP16, "c_ident")
            iotarow = load_const(IOTAROW, [P, P], FP16, "c_iotarow")
            iotacol = load_const(IOTACOL, [P, 1], F32, "c_iotacol")
            w1ext = load_const(W1EXT, [F_IN, HD + 8], FP16, "c_w1ext")
            w2ext = load_const(W2EXT, [D, HD + 8], FP16, "c_w2ext")
            recip1 = load_const(RECIP1, [P, HD], F32, "c_recip1")
            recip2 = load_const(RECIP2, [P, HD], F32, "c_recip2")
            b1 = load_const(B1, [P, HD], F32, "c_b1")
            b2 = load_const(B2, [P, HD], F32, "c_b2")
            wm1 = load_const(WM1, [D, MLP_H], FP16, "c_wm1")
            bm1 = load_const(BM1, [P, MLP_H], F32, "c_bm1")
            wm2c1 = load_const(WM2C1, [P, NCLS], FP16, "c_wm2c1")
            wm2c2 = load_const(WM2C2, [MLP_H - P, NCLS], FP16, "c_wm2c2")
            gb = load_const(GB, [P, 4], F32, "c_gb")
            bm2 = load_const(BM2, [1, NCLS], F32, "c_bm2")
            onesc = load_const(ONESC, [P, 1], FP16, "c_onesc")
            identf = load_const(IDENTF, [P, P], F32, "c_identf")
            onesf = load_const(ONESF, [P, 1], F32, "c_onesf")
            wm1f = load_const(WM1F, [D, MLP_H], F32, "c_wm1f")
            wm2c1f = load_const(WM2C1F, [P, NCLS], F32, "c_wm2c1f")
            wm2c2f = load_const(WM2C2F, [MLP_H - P, NCLS], F32, "c_wm2c2f")

            def _run_once():
                erown = singles.tile([P, NCHUNK, 4], FP16, tag="c_erown")
                nc.vector.memset(erown[:], 0.0)
                erown2 = singles.tile([P, NCHUNK, 4], FP16, tag="c_erown2")
                nc.vector.memset(erown2[:], 0.0)

                # ---------------- projection phase (full table) ----------------
                def projection(x_dram, x_dt, F, wext, table, cast):
                    with tc.tile_pool(name="proj_sb", bufs=3) as sb, \
                         tc.tile_pool(name="proj_ps", bufs=2, space="PSUM") as ps:
                        for i in range(NTILE):
                            r0 = i * P
                            rows = min(P, N - r0)
                            xt = sb.tile([P, F], FP16, tag="xt")
                            if cast:
                                nc.gpsimd.dma_start(out=xt[:rows], in_=x_dram[r0:r0 + rows, :])
                            else:
                                nc.sync.dma_start(out=xt[:rows], in_=x_dram[r0:r0 + rows, :])
                            tp = ps.tile([F, P], FP16, tag="tp")
                            nc.tensor.transpose(out=tp[:, :rows], in_=xt[:rows, :],
                                                identity=ident[:rows, :rows])
                            xT = sb.tile([F, P], FP16, tag="xT")
                            nc.vector.tensor_copy(out=xT[:, :rows], in_=tp[:, :rows])
                            hp = ps.tile([P, HD + 8], F32, tag="hp")
                            nc.tensor.matmul(hp[:rows, :], lhsT=xT[:, :rows], rhs=wext[:],
                                             start=True, stop=True)
                            rowt = sb.tile([P, ROW], FP16, tag="rowt")
                            nc.vector.tensor_copy(out=rowt[:rows, 0:HD], in_=hp[:rows, 0:HD])
                            nc.vector.tensor_copy(
                                out=rowt[:rows, ELOFF:ELOFF + 16].bitcast(F32),
                                in_=hp[:rows, HD:HD + 8])
                            nc.sync.dma_start(out=table[r0:r0 + rows, 0:ELOFF + 16],
                                              in_=rowt[:rows, 0:ELOFF + 16])

                # --------------- own-er prologue (per-chunk er table) -----------
                def er_prologue(x_dram, F, wext, dest, cast):
                    with tc.tile_pool(name="er_sb", bufs=3) as sb, \
                         tc.tile_pool(name="er_ps", bufs=2, space="PSUM") as ps:
                        for j in range(NCHUNK):
                            r0 = j * P
                            rows = min(P, NSHARD - r0)
                            xo = sb.tile([P, F], FP16, tag="xo")
                            if cast:
                                nc.gpsimd.dma_start(out=xo[:rows], in_=x_dram[r0:r0 + rows, :])
                            else:
                                nc.sync.dma_start(out=xo[:rows], in_=x_dram[r0:r0 + rows, :])
                            tp = ps.tile([F, P], FP16, tag="tp")
                            nc.tensor.transpose(out=tp[:, :rows], in_=xo[:rows, :],
                                                identity=ident[:rows, :rows])
                            xoT = sb.tile([F, P], FP16, tag="xoT")
                            nc.vector.tensor_copy(out=xoT[:, :rows], in_=tp[:, :rows])
                            ep = ps.tile([P, 4], F32, tag="ep")
                            nc.tensor.matmul(ep[:rows, :], lhsT=xoT[:, :rows],
                                             rhs=wext[:, HD + 4:HD + 8], start=True, stop=True)
                            nc.vector.tensor_copy(out=dest[:rows, j, :], in_=ep[:rows, :])

                # ------------------------- edge phase ---------------------------
                def edge_phase(table, ero, recip_c, bias_c, xout, out_f32=False):
                    nch = NCHUNK if max_chunks is None else min(max_chunks, NCHUNK)
                    with tc.tile_pool(name="eg", bufs=2) as eg, \
                         tc.tile_pool(name="em", bufs=3) as em, \
                         tc.tile_pool(name="es", bufs=2) as es_pool, \
                         tc.tile_pool(name="eps", bufs=2, space="PSUM") as eps:
                        toff = 0
                        for j in range(nch):
                            Tl, Th = T_lo[j], T_hi[j]
                            T = Tl + Th
                            rows = min(P, NSHARD - j * P)
                            dlc = es_pool.tile([P, T], F32, tag="dlc")
                            nc.sync.dma_start(out=dlc[:], in_=DLC[:, toff:toff + T])
                            idxt = es_pool.tile([P, 8 * T], I16, tag="idxt")
                            nc.sync.dma_start(out=idxt[:], in_=IDX[:, 8 * toff:8 * (toff + T)])
                            gbuf = eg.tile([P, T, ROW], FP16, tag="gbuf")
                            nc.gpsimd.dma_gather(
                                out_ap=gbuf[:, 0:Tl, :], in_ap=table[0:SPLIT, :],
                                idxs_ap=idxt[:, 0:8 * Tl], num_idxs=P * Tl,
                                num_idxs_reg=P * Tl, elem_size=ROW, single_packet=False)
                            if Th:
                                nc.gpsimd.dma_gather(
                                    out_ap=gbuf[:, Tl:T, :], in_ap=table[SPLIT:N, :],
                                    idxs_ap=idxt[:, 8 * Tl:8 * T], num_idxs=P * Th,
                                    num_idxs_reg=P * Th, elem_size=ROW, single_packet=False)
                            if ev == 1:
                                xo16 = es_pool.tile([P, D], FP16, tag="xo16")
                                nc.vector.tensor_copy(out=xo16[:], in_=gbuf[:, 0, 0:D])
                                nc.sync.dma_start(out=xout[j * P:j * P + rows, :],
                                                  in_=xo16[:rows, :])
                                toff += T
                                continue
                            # batched mask-transpose build: ONE broadcast DMA + ONE compare
                            mtb = em.tile([P, T * P], FP16, tag="mtb")
                            dlr_sl = DLR[0:1, toff * P:(toff + T) * P]
                            nc.gpsimd.dma_start(out=mtb[:], in_=bass.AP(
                                tensor=dlr_sl.tensor, offset=dlr_sl.offset,
                                ap=[[0, P]] + dlr_sl.ap[1:]))
                            mt = em.tile([P, T * P], FP16, tag="mt")
                            nc.vector.tensor_scalar(out=mt[:], in0=mtb[:],
                                                    scalar1=iotacol[:], scalar2=None,
                                                    op0=ALU.is_equal)
                            # batched forward masks: M[p,(t,c)] = (iota_row[c] == dlc[p,t])
                            m_all = em.tile([P, T, P], FP16, tag="m_all")
                            ir_ap = iotarow[:]
                            ir_tiled = bass.AP(tensor=ir_ap.tensor, offset=ir_ap.offset,
                                               ap=[ir_ap.ap[0], [0, T], ir_ap.ap[1]])
                            dlc_ap = dlc[:]
                            dlc_b = bass.AP(tensor=dlc_ap.tensor, offset=dlc_ap.offset,
                                            ap=[dlc_ap.ap[0], dlc_ap.ap[1], [0, P]])
                            nc.vector.tensor_tensor(out=m_all[:], in0=ir_tiled, in1=dlc_b,
                                                    op=ALU.is_equal)
                            # er per edge-slot: T small matmuls into one PSUM strip
                            erps = eps.tile([P, T * 4], F32, tag="erp")
                            for t in range(T):
                                nc.tensor.matmul(erps[:, t * 4:(t + 1) * 4],
                                                 lhsT=mt[:, t * P:(t + 1) * P],
                                                 rhs=ero[:, j, :], start=True, stop=True)
                            # e = el + er  (batched), LeakyReLU, exp
                            e_sb = es_pool.tile([P, T, 4], F32, tag="e_sb")
                            el_view = gbuf[:, :, ELOFF:ELOFF + 8].bitcast(F32)[:, :, 0:4]
                            nc.vector.tensor_tensor(out=e_sb[:], in0=el_view,
                                                    in1=erps[:].rearrange("p (t h) -> p t h", h=4),
                                                    op=ALU.add)
                            lr = es_pool.tile([P, T, 4], F32, tag="lr")
                            nc.vector.tensor_scalar(out=lr[:], in0=e_sb[:], scalar1=NEG,
                                                    scalar2=None, op0=ALU.mult)
                            nc.vector.tensor_tensor(out=lr[:], in0=e_sb[:], in1=lr[:],
                                                    op=ALU.max)
                            ex = es_pool.tile([P, T, 4], F32, tag="ex")
                            nc.scalar.activation(ex[:], lr[:], ACTF.Exp)
                            nc.vector.tensor_copy(out=gbuf[:, :, 272:276], in_=ex[:])
                            # scale gathered features by ex (broadcast over d) in one op
                            gb0 = gbuf[:, 0, 0:HD]
                            hv_all = bass.AP(tensor=gb0.tensor, offset=gb0.offset,
                                             ap=[gb0.ap[0], [ROW, T], [D, H], [1, D]])
                            gex0 = gbuf[:, 0, 272:276]
                            ex_b = bass.AP(tensor=gex0.tensor, offset=gex0.offset,
                                           ap=[gex0.ap[0], [ROW, T], [1, 4], [0, D]])
                            nc.vector.tensor_tensor(out=hv_all, in0=hv_all, in1=ex_b,
                                                    op=ALU.mult)

                            agg = eps.tile([P, HD + 20], F32, tag="agg")
                            for t in range(T):
                                nc.tensor.matmul(agg[:], lhsT=m_all[:, t, :],
                                                 rhs=gbuf[:, t, 0:HD + 20],
                                                 start=(t == 0), stop=(t == T - 1))
                            s_sb = es_pool.tile([P, 4], F32, tag="s_sb")
                            nc.vector.tensor_copy(out=s_sb[:], in_=agg[:, HD + 16:HD + 20])
                            sr = es_pool.tile([P, 4], F32, tag="sr")
                            nc.vector.reciprocal(sr[:], s_sb[:])
                            osb = es_pool.tile([P, HD], F32, tag="osb")
                            for h in range(H):
                                nc.vector.tensor_scalar_mul(osb[:, h * D:(h + 1) * D],
                                                            agg[:, h * D:(h + 1) * D],
                                                            sr[:, h:h + 1])
                            nc.vector.tensor_tensor(out=osb[:], in0=osb[:], in1=recip_c[:],
                                                    op=ALU.mult)
                            nc.vector.tensor_tensor(out=osb[:], in0=osb[:], in1=bias_c[:],
                                                    op=ALU.add)
                            nc.scalar.activation(osb[:], osb[:], ACTF.Relu)
                            xo = es_pool.tile([P, D], F32, tag="xo2")
                            nc.vector.tensor_reduce(
                                out=xo[:], in_=osb[:].rearrange("p (h d) -> p d h", h=H),
                                axis=mybir.AxisListType.X, op=ALU.add)
                            if out_f32:
                                xof = es_pool.tile([P, D], F32, tag="xof")
                                nc.scalar.activation(xof[:], xo[:], ACTF.Copy, scale=0.25)
                                nc.sync.dma_start(out=xout[j * P:j * P + rows, :],
                                                  in_=xof[:rows, :])
                            else:
                                xo16 = es_pool.tile([P, D], FP16, tag="xo16")
                                nc.scalar.activation(xo16[:], xo[:], ACTF.Copy, scale=0.25)
                                nc.sync.dma_start(out=xout[j * P:j * P + rows, :],
                                                  in_=xo16[:rows, :])
                            toff += T

                # ------------------------------ go ------------------------------
                order = ["P1", "E1", "AG", "P2", "E2", "full"]
                upto = order.index(phases if phases != "full" else "full")
                done = False

                projection(features, F32, F_IN, w1ext, table1, cast=False)
                er_prologue(fown, F_IN, w1ext, erown, cast=False)
                done = upto <= order.index("P1")
                if not done:
                    edge_phase(table1, erown, recip1, b1, x2slice)
                    done = upto <= order.index("E1")
                if not done:
                    nc.gpsimd.collective_compute(
                        "AllGather", mybir.AluOpType.bypass,
                        replica_groups=[list(range(NCORES))],
                        ins=[x2slice[:]], outs=[x2full[:]])
                    done = upto <= order.index("AG")
                if not done:
                    projection(x2full, FP16, D, w2ext, table2, cast=False)
                    er_prologue(x2slice, D, w2ext, erown2, cast=False)
                    edge_phase(table2, erown2, recip2, b2, x3slice, out_f32=True)
                    done = upto <= order.index("E2")
                skip_mlp = done
                if skip_mlp:
                    with tc.tile_pool(name="dbg0", bufs=1) as dbg0:
                        z = dbg0.tile([P, NCLS], F32, tag="dbgz")
                        nc.vector.memset(z[:], 0.0)
                        for j in range(NCHUNK):
                            r0 = j * P
                            rows = min(P, NSHARD - r0)
                            nc.sync.dma_start(out=out[r0:r0 + rows, :], in_=z[:rows])

                # ------------------------------ MLP -----------------------------
                if skip_mlp:
                    raise _SkipRest()

                SCALE_ZC = 512.0

                def zproj(m_pool, p_pool, j, rows):
                    """Recompute z (f32) for chunk j -> sbuf tile [P, MLP_H]."""
                    x3 = m_pool.tile([P, D], F32, tag="x3")
                    nc.sync.dma_start(out=x3[:rows], in_=x3slice[j * P:j * P + rows, :])
                    tp = p_pool.tile([D, P], F32, tag="tp")
                    nc.tensor.transpose(out=tp[:, :rows], in_=x3[:rows, :],
                                        identity=identf[:rows, :rows])
                    x3T = m_pool.tile([D, P], F32, tag="x3T")
                    nc.vector.tensor_copy(out=x3T[:, :rows], in_=tp[:, :rows])
                    zp = p_pool.tile([P, MLP_H], F32, tag="zp")
                    nc.tensor.matmul(zp[:rows, :], lhsT=x3T[:, :rows], rhs=wm1f[:],
                                     start=True, stop=True)
                    zsb = m_pool.tile([P, MLP_H], F32, tag="zsb")
                    nc.vector.tensor_tensor(out=zsb[:rows], in0=zp[:rows], in1=bm1[:rows],
                                            op=ALU.add)
                    nc.scalar.activation(zsb[:rows], zsb[:rows], ACTF.Relu)
                    return zsb

                # ---- pass A: global feature means ----
                with tc.tile_pool(name="ma", bufs=3) as ma, \
                     tc.tile_pool(name="map", bufs=2, space="PSUM") as map_, \
                     tc.tile_pool(name="sta", bufs=1, space="PSUM") as sta:
                    sa1 = sta.tile([P, 1], F32, tag="sa1")
                    sa2 = sta.tile([P, 1], F32, tag="sa2")
                    for j in range(NCHUNK):
                        rows = min(P, NSHARD - j * P)
                        zsb = zproj(ma, map_, j, rows)
                        first, last = (j == 0), (j == NCHUNK - 1)
                        nc.tensor.matmul(sa1[:], lhsT=zsb[:rows, 0:P], rhs=onesf[:rows],
                                         start=first, stop=last)
                        nc.tensor.matmul(sa2[:MLP_H - P], lhsT=zsb[:rows, P:MLP_H],
                                         rhs=onesf[:rows], start=first, stop=last)
                    pk = ma.tile([P, 2], F32, tag="pk")
                    nc.vector.memset(pk[:], 0.0)
                    nc.vector.tensor_copy(out=pk[:, 0:1], in_=sa1[:])
                    nc.vector.tensor_copy(out=pk[:MLP_H - P, 1:2], in_=sa2[:MLP_H - P])
                    nc.sync.dma_start(out=ccin1[:], in_=pk[:])

                nc.gpsimd.collective_compute(
                    "AllReduce", mybir.AluOpType.add,
                    replica_groups=[list(range(NCORES))],
                    ins=[ccin1[:]], outs=[ccout1[:]])

                # ---- pass B: centered scaled z + variance stats ----
                zstore = singles.tile([P, NCHUNK, MLP_H + 1], F32, tag="c_zstore")
                nc.vector.memset(zstore[:], 0.0)
                with tc.tile_pool(name="mb", bufs=3) as mb, \
                     tc.tile_pool(name="mbp", bufs=2, space="PSUM") as mbp, \
                     tc.tile_pool(name="stb", bufs=1, space="PSUM") as stb:
                    mus = mb.tile([P, 2], F32, tag="mus")
                    nc.sync.dma_start(out=mus[:], in_=ccout1[:])
                    nc.vector.tensor_scalar(out=mus[:], in0=mus[:], scalar1=1.0 / N,
                                            scalar2=None, op0=ALU.mult)
                    nc.sync.dma_start(out=muraw[0, 0:P], in_=mus[:, 0:1])
                    nc.sync.dma_start(out=muraw[0, P:MLP_H], in_=mus[:MLP_H - P, 1:2])
                    mub = mb.tile([P, MLP_H], F32, tag="mub")
                    mu_row = muraw[0:1, :]
                    nc.gpsimd.dma_start(out=mub[:], in_=bass.AP(
                        tensor=mu_row.tensor, offset=mu_row.offset,
                        ap=[[0, P]] + mu_row.ap[1:]))
                    sb1 = stb.tile([P, 1], F32, tag="sb1")
                    sb2 = stb.tile([P, 1], F32, tag="sb2")
                    sq1 = stb.tile([P, 1], F32, tag="sq1")
                    sq2 = stb.tile([P, 1], F32, tag="sq2")
                    for j in range(NCHUNK):
                        rows = min(P, NSHARD - j * P)
                        zsb = zproj(mb, mbp, j, rows)
                        zc = zstore[:, j, 0:MLP_H]
                        nc.vector.tensor_tensor(out=zc[:rows], in0=zsb[:rows],
                                                in1=mub[:rows], op=ALU.subtract)
                        nc.vector.tensor_scalar(out=zc[:rows], in0=zc[:rows],
                                                scalar1=SCALE_ZC, scalar2=None,
                                                op0=ALU.mult)
                        nc.vector.memset(zstore[:rows, j, MLP_H:MLP_H + 1], 1.0)
                        zq = mb.tile([P, MLP_H], F32, tag="zq")
                        nc.vector.tensor_tensor(out=zq[:rows], in0=zc[:rows], in1=zc[:rows],
                                                op=ALU.mult)
                        first, last = (j == 0), (j == NCHUNK - 1)
                        nc.tensor.matmul(sb1[:], lhsT=zc[:rows, 0:P], rhs=onesf[:rows],
                                         start=first, stop=last)
                        nc.tensor.matmul(sb2[:MLP_H - P], lhsT=zc[:rows, P:MLP_H],
                                         rhs=onesf[:rows], start=first, stop=last)
                        nc.tensor.matmul(sq1[:], lhsT=zq[:rows, 0:P], rhs=onesf[:rows],
                                         start=first, stop=last)
                        nc.tensor.matmul(sq2[:MLP_H - P], lhsT=zq[:rows, P:MLP_H],
                                         rhs=onesf[:rows], start=first, stop=last)
                    pk2 = mb.tile([P, 4], F32, tag="pk2")
                    nc.vector.memset(pk2[:], 0.0)
                    nc.vector.tensor_copy(out=pk2[:, 0:1], in_=sb1[:])
                    nc.vector.tensor_copy(out=pk2[:MLP_H - P, 1:2], in_=sb2[:MLP_H - P])
                    nc.vector.tensor_copy(out=pk2[:, 2:3], in_=sq1[:])
                    nc.vector.tensor_copy(out=pk2[:MLP_H - P, 3:4], in_=sq2[:MLP_H - P])
                    nc.sync.dma_start(out=ccin[:], in_=pk2[:])

                nc.gpsimd.collective_compute(
                    "AllReduce", mybir.AluOpType.add,
                    replica_groups=[list(range(NCORES))],
                    ins=[ccin[:]], outs=[ccout[:]])

                # ---- pass C: BN constants + folded final weights ----
                with tc.tile_pool(name="m2", bufs=3) as m2, \
                     tc.tile_pool(name="m2p", bufs=2, space="PSUM") as m2p:
                    stg = m2.tile([P, 4], F32, tag="stg")
                    nc.sync.dma_start(out=stg[:], in_=ccout[:])
                    m1t = m2.tile([P, 2], F32, tag="m1t")   # mean of zc (scaled units)
                    nc.vector.tensor_scalar(out=m1t[:], in0=stg[:, 0:2], scalar1=1.0 / N,
                                            scalar2=None, op0=ALU.mult)
                    m2t = m2.tile([P, 2], F32, tag="m2t")   # mean of zc^2 (scaled units)
                    nc.vector.tensor_scalar(out=m2t[:], in0=stg[:, 2:4], scalar1=1.0 / N,
                                            scalar2=None, op0=ALU.mult)
                    var = m2.tile([P, 2], F32, tag="var")
                    nc.vector.tensor_tensor(out=var[:], in0=m1t[:], in1=m1t[:], op=ALU.mult)
                    nc.vector.tensor_tensor(out=var[:], in0=m2t[:], in1=var[:],
                                            op=ALU.subtract)
                    nc.vector.tensor_scalar(out=var[:], in0=var[:],
                                            scalar1=1.0 / (SCALE_ZC * SCALE_ZC),
                                            scalar2=None, op0=ALU.mult)
                    nc.vector.tensor_scalar(out=var[:], in0=var[:], scalar1=EPS,
                                            scalar2=None, op0=ALU.add)
                    std = m2.tile([P, 2], F32, tag="std")
                    nc.scalar.activation(std[:], var[:], ACTF.Sqrt)
                    rstd = m2.tile([P, 2], F32, tag="rstd")
                    nc.vector.reciprocal(rstd[:], std[:])
                    # g2 applies to zc_s: g2 = gamma * rstd / SCALE_ZC
                    gp = m2.tile([P, 2], F32, tag="gp")
                    nc.vector.tensor_tensor(out=gp[:], in0=gb[:, 0:2], in1=rstd[:],
                                            op=ALU.mult)
                    nc.vector.tensor_scalar(out=gp[:], in0=gp[:], scalar1=1.0 / SCALE_ZC,
                                            scalar2=None, op0=ALU.mult)
                    # b2c = beta - m1 * g2   (m1 in scaled units)
                    bp = m2.tile([P, 2], F32, tag="bp")
                    nc.vector.tensor_tensor(out=bp[:], in0=m1t[:], in1=gp[:], op=ALU.mult)
                    nc.vector.tensor_tensor(out=bp[:], in0=gb[:, 2:4], in1=bp[:],
                                            op=ALU.subtract)
                    wp1 = m2.tile([P, NCLS], F32, tag="wp1")
                    nc.vector.tensor_scalar_mul(wp1[:], wm2c1f[:], gp[:, 0:1])
                    wp2 = m2.tile([P, NCLS], F32, tag="wp2")
                    nc.vector.memset(wp2[:], 0.0)
                    nc.vector.tensor_scalar_mul(wp2[:MLP_H - P, :], wm2c2f[:],
                                                gp[:MLP_H - P, 1:2])
                    cp = m2p.tile([1, NCLS], F32, tag="cp")
                    nc.tensor.matmul(cp[:], lhsT=bp[:, 0:1], rhs=wm2c1f[:],
                                     start=True, stop=False)
                    nc.tensor.matmul(cp[:], lhsT=bp[:MLP_H - P, 1:2], rhs=wm2c2f[:],
                                     start=False, stop=True)
                    cps = m2.tile([1, NCLS], F32, tag="cps")
                    nc.vector.tensor_tensor(out=cps[:], in0=cp[:], in1=bm2[:], op=ALU.add)
                    nc.sync.dma_start(out=wp2[MLP_H - P:MLP_H - P + 1, :], in_=cps[:])

                    # ---- pass D: out = zc_s @ W'' (+ c'' via ones row) ----
                    for j in range(NCHUNK):
                        r0 = j * P
                        rows = min(P, NSHARD - r0)
                        t1 = m2p.tile([P, P], F32, tag="t1")
                        nc.tensor.transpose(out=t1[:], in_=zstore[:, j, 0:P],
                                            identity=identf[:])
                        zT1 = m2.tile([P, P], F32, tag="zT1")
                        nc.vector.tensor_copy(out=zT1[:], in_=t1[:])
                        t2 = m2p.tile([MLP_H + 1 - P, P], F32, tag="t2")
                        nc.tensor.transpose(out=t2[:], in_=zstore[:, j, P:MLP_H + 1],
                                            identity=identf[:])
                        zT2 = m2.tile([MLP_H + 1 - P, P], F32, tag="zT2")
                        nc.vector.tensor_copy(out=zT2[:], in_=t2[:])
                        op_ = m2p.tile([P, NCLS], F32, tag="op")
                        nc.tensor.matmul(op_[:], lhsT=zT1[:], rhs=wp1[:],
                                         start=True, stop=False)
                        nc.tensor.matmul(op_[:], lhsT=zT2[:], rhs=wp2[:MLP_H + 1 - P, :],
                                         start=False, stop=True)
                        ofin = m2.tile([P, NCLS], F32, tag="ofin")
                        nc.vector.tensor_copy(out=ofin[:rows], in_=op_[:rows])
                        nc.sync.dma_start(out=out[r0:r0 + rows, :], in_=ofin[:rows])

            for _rep in range(reps):
                _run_once()

        except _SkipRest:
            pass
    nc.finalize()
    return nc


# ----------------------------------------------------------------------------
# Host entry
# ----------------------------------------------------------------------------

def prep_inputs(inputs):
    """Returns (plan, in_maps)."""
    f32 = np.float32
    W1ext, recip1 = _fold_weights(np.asarray(inputs["W1"], f32),
                                  np.asarray(inputs["al1"], f32),
                                  np.asarray(inputs["ar1"], f32))
    W2ext, recip2 = _fold_weights(np.asarray(inputs["W2"], f32),
                                  np.asarray(inputs["al2"], f32),
                                  np.asarray(inputs["ar2"], f32))
    plan, earrays = _prep_edges(inputs["src"], inputs["dst"])

    features = np.ascontiguousarray(np.asarray(inputs["features"], f32).astype(np.float16))
    ident = np.eye(P, dtype=np.float16)
    iotarow = np.broadcast_to(np.arange(P, dtype=np.float16), (P, P)).copy()
    iotacol = np.arange(P, dtype=f32).reshape(P, 1)
    bc = lambda v: np.broadcast_to(np.asarray(v, f32).reshape(1, -1), (P, len(np.asarray(v).reshape(-1)))).copy()
    consts = {
        "IDENT": ident, "IOTAROW": iotarow, "IOTACOL": iotacol,
        "W1EXT": W1ext, "W2EXT": W2ext,
        "RECIP1": bc(recip1), "RECIP2": bc(recip2),
        "B1": bc(inputs["b1"]), "B2": bc(inputs["b2"]),
        "WM1": np.asarray(inputs["Wm1"], np.float16),
        "BM1": bc(inputs["bm1"]),
        "WM2C1": np.asarray(inputs["Wm2"], np.float16)[0:P, :],
        "WM2C2": np.asarray(inputs["Wm2"], np.float16)[P:MLP_H, :],
        "BM2": np.asarray(inputs["bm2"], f32).reshape(1, NCLS),
        "ONESC": np.ones((P, 1), np.float16),
        "IDENTF": np.eye(P, dtype=f32),
        "ONESF": np.ones((P, 1), f32),
        "WM1F": np.asarray(inputs["Wm1"], f32),
        "WM2C1F": np.asarray(inputs["Wm2"], f32)[0:P, :],
        "WM2C2F": np.asarray(inputs["Wm2"], f32)[P:MLP_H, :],
    }
    gbarr = np.zeros((P, 4), f32)
    gamma = np.asarray(inputs["gamma"], f32)
    beta = np.asarray(inputs["beta"], f32)
    gbarr[:, 0] = gamma[0:P]
    gbarr[0:MLP_H - P, 1] = gamma[P:MLP_H]
    gbarr[:, 2] = beta[0:P]
    gbarr[0:MLP_H - P, 3] = beta[P:MLP_H]
    consts["GB"] = gbarr

    in_maps = []
    for c in range(NCORES):
        IDXa, DLCa, DLRa = earrays[c]
        m = dict(consts)
        m["features"] = features
        m["fown"] = features[c * NSHARD:(c + 1) * NSHARD, :]
        m["IDX"] = IDXa
        m["DLC"] = DLCa
        m["DLR"] = DLRa
        in_maps.append(m)
    return plan, in_maps


def kernel(**inputs):
    from concourse.bass_utils import run_bass_kernel_spmd
    plan, in_maps = prep_inputs(inputs)
    nc = build_nc(plan)
    res = run_bass_kernel_spmd(nc, in_maps, core_ids=list(range(NCORES)))
    out = np.concatenate([res.results[c]["out"] for c in range(NCORES)], axis=0)
    return out.astype(np.float32)


if __name__ == "__main__":
    import time
    t0 = time.time()
    plan, _ = _prep_edges(
        np.concatenate([np.random.randint(0, N, 800000), np.arange(N)]).astype(np.int32),
        np.concatenate([np.random.randint(0, N, 800000), np.arange(N)]).astype(np.int32))
    print("edge prep:", time.time() - t0, "totT:", plan["totT"])
    t0 = time.time()
    nc = build_nc(plan)
    print("build:", time.time() - t0)



# revision 2
# speedup vs baseline: 1.3884x; 1.3884x over previous
"""GAT (2-layer, 4-head) + MLP/BatchNorm predictor on 8 Trainium2 NeuronCores.

v2 strategy (graph-parallel, dst-sharded; engine-balanced):
  - Nodes split contiguously: core c owns dsts [c*6250, (c+1)*6250). Edges live
    with their dst core, sorted by dst, grouped into 49 chunks of <=128 dsts.
  - Features are uploaded TRANSPOSED (featT [128, N] fp16) so the projection
    needs no on-chip transposes: per 128-row tile one fp16 matmul
    x @ [W | Wel | Wer] -> PSUM f32, cast f32->fp16 round-robin on
    DVE/Act/Pool, batched 4-tile DMA into the table (row = 256 h fp16
    (d,h)-interleaved | el 4xfp16 | er 4xfp16 | pad to 768B).
  - Edge phase per chunk: dma_gather of 768B src rows; dst-mask m_all built by
    one DVE is_equal (all-fp16 packed APs -> 2x mode); transpose-mask mt via
    PE transposes of m_all + one DVE copy; er broadcast dst->slot via tiny
    mt matmuls; e = el+er, LeakyReLU on DVE, exp on Act (only Act func in the
    phase -> no activation-table reloads) written straight into the row pad;
    h *= ex as ONE DVE multiply at 2x rate thanks to the (d,h) interleave;
    aggregation + softmax denominators via T accumulating mask matmuls on PE;
    node-space epilogue (1/s scaling, bias, ReLU, head-mean) on GpSimd
    scalar_tensor_tensor ops; output transposed on PE into SBUF-resident
    x2T/x3T slabs.
  - Collectives: AllGather of x2T (fp16, [64, 6250] per core) so each core
    projects the full layer-2 table; single AllReduce of BatchNorm raw
    moment sums (single-pass E[z], E[z^2] in f32).
"""
import sys

sys.path.insert(0, "/opt/trn_rl_repo")

import numpy as np

N = 50000
F_IN = 128
H = 4
D = 64
HD = 256
NCORES = 8
NSHARD = N // NCORES          # 6250
P = 128
NCHUNK = (NSHARD + P - 1) // P  # 49 (last chunk 106 dsts)
SPLIT = 32768                 # int16 gather index limit
MLP_H = 200
NCLS = 2
NEG = 0.2
EPS = 1e-5
ROW = 384                     # fp16 slots per table row (768 B)
ELOFF = 256                   # el at slots 256:260 (fp16), er at 260:264
EROFF = 260


def configure(n, split=32768):
    global N, NSHARD, NCHUNK, SPLIT
    N = n
    NSHARD = N // NCORES
    NCHUNK = (NSHARD + P - 1) // P
    SPLIT = split


# ----------------------------------------------------------------------------
# Host-side preprocessing
# ----------------------------------------------------------------------------

_PERM = None


def _perm():
    """Column permutation (h,d) -> (d,h): new[d*H+h] = old[h*D+d]."""
    global _PERM
    if _PERM is None:
        idx = np.arange(HD).reshape(H, D)          # old[h, d]
        _PERM = idx.T.reshape(-1)                  # new[(d, h)]
    return _PERM


def _fold_weights(W, al, ar):
    """W:[F,H*D] al,ar:[H,D] -> Wext [F, HD+8] f16 with (d,h) interleave."""
    F = W.shape[0]
    W64 = W.astype(np.float64)
    Wel = (W64.reshape(F, H, D) * al[None].astype(np.float64)).sum(-1)  # [F,H]
    Wer = (W64.reshape(F, H, D) * ar[None].astype(np.float64)).sum(-1)
    Wr = W64[:, _perm()]
    return np.concatenate([Wr, Wel, Wer], axis=1).astype(np.float16)


def _prep_edges(src, dst):
    """Per-core gather arrays. Returns (plan, per_core arrays).

    plan: T_lo[j], T_hi[j], totT, TMAX (identical across cores).
    per-core: IDX [128, 8*totT] i16, DLC [128, totT] f16 (pad -1).
    """
    src = np.asarray(src)
    dst = np.asarray(dst)
    per_core = []
    for c in range(NCORES):
        m = (dst >= c * NSHARD) & (dst < (c + 1) * NSHARD)
        es, ed = src[m], dst[m] - c * NSHARD
        order = np.argsort(ed, kind="stable")
        es, ed = es[order], ed[order]
        starts = np.searchsorted(ed, np.arange(0, NCHUNK * P, P))
        ends = np.searchsorted(ed, np.minimum(np.arange(P, (NCHUNK + 1) * P, P), NSHARD))
        chunks = []
        for j in range(NCHUNK):
            cs, ce = starts[j], ends[j]
            s_j, d_j = es[cs:ce], ed[cs:ce] - j * P
            lo = s_j < SPLIT
            chunks.append((s_j[lo], d_j[lo], s_j[~lo] - SPLIT, d_j[~lo]))
        per_core.append(chunks)

    T_lo = np.zeros(NCHUNK, np.int64)
    T_hi = np.zeros(NCHUNK, np.int64)
    for c in range(NCORES):
        for j in range(NCHUNK):
            slo, _, shi, _ = per_core[c][j]
            T_lo[j] = max(T_lo[j], -(-len(slo) // P))
            T_hi[j] = max(T_hi[j], -(-len(shi) // P))
    T_lo = np.maximum(T_lo, 1)
    totT = int((T_lo + T_hi).sum())
    TMAX = int((T_lo + T_hi).max())

    def wrap_idx(flat):
        n = len(flat)
        cols = n // 16
        a = flat.reshape(cols, 16).T.astype(np.int16)      # [16, cols]
        return np.tile(a, (8, 1))                          # [128, cols]

    arrays = []
    for c in range(NCORES):
        idx_cols = []
        dlc = np.full((P, totT), -1.0, np.float16)
        t0 = 0
        for j in range(NCHUNK):
            slo, dlo, shi, dhi = per_core[c][j]
            for (s_j, d_j, T) in ((slo, dlo, T_lo[j]), (shi, dhi, T_hi[j])):
                nslot = int(T) * P
                if nslot == 0:
                    continue
                idx = np.zeros(nslot, np.int16)
                dl = np.full(nslot, -1.0, np.float32)
                idx[: len(s_j)] = s_j
                dl[: len(s_j)] = d_j
                idx_cols.append(wrap_idx(idx))
                dlc[:, t0 : t0 + int(T)] = dl.reshape(int(T), P).T.astype(np.float16)
                t0 += int(T)
        assert t0 == totT
        IDX = np.concatenate(idx_cols, axis=1)
        assert IDX.shape == (P, 8 * totT)
        arrays.append((IDX, dlc))

    plan = {"T_lo": T_lo.tolist(), "T_hi": T_hi.tolist(), "totT": totT,
            "TMAX": TMAX}
    return plan, arrays


# ----------------------------------------------------------------------------
# Bass program
# ----------------------------------------------------------------------------

def build_nc(plan, phases='full', reps=1, max_chunks=None):
    import concourse.bacc as bacc
    import concourse.bass as bass
    import concourse.tile as tile
    from concourse import mybir

    FP16 = mybir.dt.float16
    F32 = mybir.dt.float32
    I16 = mybir.dt.int16
    ALU = mybir.AluOpType
    ACTF = mybir.ActivationFunctionType

    T_lo, T_hi, totT = plan["T_lo"], plan["T_hi"], plan["totT"]
    TMAX = plan["TMAX"]
    NTILE = (N + P - 1) // P            # 391 (last 80 rows)
    WCOLS = HD + 8

    nc = bacc.Bacc("TRN2", target_bir_lowering=False, debug=False,
                   num_devices=NCORES)

    dp = lambda name, shape, dt: nc.declare_dram_parameter(name, shape, dt, isOutput=False)
    FEATT = dp("FEATT", [P, N], FP16)
    FOWNT = dp("FOWNT", [P, NSHARD], FP16)
    IDX = dp("IDX", [P, 8 * totT], I16)
    DLC = dp("DLC", [P, totT], FP16)
    IOTAW = dp("IOTAW", [P, P * TMAX], FP16)
    IDENT = dp("IDENT", [P, P], FP16)
    IDENTF = dp("IDENTF", [P, P], F32)
    W1EXT = dp("W1EXT", [F_IN, WCOLS], FP16)
    W2EXT = dp("W2EXT", [D, WCOLS], FP16)
    B1Q = dp("B1Q", [P, HD], F32)
    B2Q = dp("B2Q", [P, HD], F32)
    WM1F = dp("WM1F", [D, MLP_H], F32)
    BM1 = dp("BM1", [P, MLP_H], F32)
    WM2C1F = dp("WM2C1F", [P, NCLS], F32)
    WM2C2F = dp("WM2C2F", [MLP_H - P, NCLS], F32)
    GB = dp("GB", [P, 4], F32)
    BM2 = dp("BM2", [1, NCLS], F32)
    ONESF = dp("ONESF", [P, 1], F32)

    out = nc.declare_dram_parameter("out", [NSHARD, NCLS], F32, isOutput=True)

    table1 = nc.dram_tensor("table1", [N, ROW], FP16)
    table2 = nc.dram_tensor("table2", [N, ROW], FP16)
    x2sliceT = nc.dram_tensor("x2sliceT", [D, NSHARD], FP16)
    x2fullT = nc.dram_tensor("x2fullT", [NCORES * D, NSHARD], FP16,
                             addr_space="Shared")
    ccin = nc.dram_tensor("ccin", [P, 4], F32)
    ccout = nc.dram_tensor("ccout", [P, 4], F32, addr_space="Shared")

    class _SkipRest(Exception):
        pass

    with tile.TileContext(nc) as tc:
        import contextlib
        try:
          with contextlib.ExitStack() as ctx:
            singles = ctx.enter_context(tc.tile_pool(name="singles", bufs=1))

            def load_const(param, shape, dtype, tag):
                t = singles.tile(shape, dtype, tag=tag)
                nc.sync.dma_start(out=t[:], in_=param[:])
                return t

            identb = load_const(IDENT, [P, P], FP16, "c_ident")
            identf = load_const(IDENTF, [P, P], F32, "c_identf")
            iotaw = load_const(IOTAW, [P, P, TMAX], FP16, "c_iotaw")
            w1ext = load_const(W1EXT, [F_IN, WCOLS], FP16, "c_w1ext")
            w2ext = load_const(W2EXT, [D, WCOLS], FP16, "c_w2ext")
            b1q = load_const(B1Q, [P, HD], F32, "c_b1q")
            b2q = load_const(B2Q, [P, HD], F32, "c_b2q")
            wm1f = load_const(WM1F, [D, MLP_H], F32, "c_wm1f")
            bm1 = load_const(BM1, [P, MLP_H], F32, "c_bm1")
            wm2c1f = load_const(WM2C1F, [P, NCLS], F32, "c_wm2c1f")
            wm2c2f = load_const(WM2C2F, [MLP_H - P, NCLS], F32, "c_wm2c2f")
            gb = load_const(GB, [P, 4], F32, "c_gb")
            bm2 = load_const(BM2, [1, NCLS], F32, "c_bm2")
            onesf = load_const(ONESF, [P, 1], F32, "c_onesf")
            fownt = load_const(FOWNT, [P, NSHARD], FP16, "c_fownt")
            idx_sb = load_const(IDX, [P, 8 * totT], I16, "c_idx")
            dlc_sb = load_const(DLC, [P, totT], FP16, "c_dlc")

            x2t_sb = singles.tile([D, NSHARD], FP16, tag="c_x2t")
            x3t_sb = singles.tile([D, NSHARD], F32, tag="c_x3t")
            erown1 = singles.tile([P, NCHUNK, 4], FP16, tag="c_erown1")
            erown2 = singles.tile([P, NCHUNK, 4], FP16, tag="c_erown2")
            zstore = singles.tile([P, NCHUNK, MLP_H + 1], F32, tag="c_zstore")

            def _run_once():
                nc.vector.memset(erown1[:], 0.0)
                nc.vector.memset(erown2[:], 0.0)
                nc.vector.memset(zstore[:], 0.0)
                # ones column (col MLP_H) for the pass-D folded-constant row
                nc.vector.memset(zstore[:, :, MLP_H:MLP_H + 1], 1.0)

                # ---------------- projection phase (full table) --------------
                def projection(layer):
                    """layer 1: featT -> table1; layer 2: x2fullT -> table2."""
                    F = F_IN if layer == 1 else D
                    wext = w1ext if layer == 1 else w2ext
                    table = table1 if layer == 1 else table2
                    SLAB = 8                     # tiles per load slab
                    BST = 8                      # tiles per store batch
                    with tc.tile_pool(name="proj_sb", bufs=3) as sb, \
                         tc.tile_pool(name="proj_st", bufs=2) as stp, \
                         tc.tile_pool(name="proj_ps", bufs=4, space="PSUM") as ps:
                        nslab = (NTILE + SLAB - 1) // SLAB
                        cast_i = 0
                        for s in range(nslab):
                            r0 = s * SLAB * P
                            ncols = min(SLAB * P, N - r0)
                            slab = sb.tile([F, SLAB * P], FP16, tag="slab")
                            if layer == 1:
                                nc.sync.dma_start(out=slab[:, 0:ncols],
                                                  in_=FEATT[:, r0:r0 + ncols])
                            else:
                                # x2fullT rows live in per-core 64-row bands
                                lo = r0
                                while lo < r0 + ncols:
                                    c = lo // NSHARD
                                    hi = min(r0 + ncols, (c + 1) * NSHARD)
                                    nc.sync.dma_start(
                                        out=slab[:, lo - r0:hi - r0],
                                        in_=x2fullT[c * D:(c + 1) * D,
                                                    lo - c * NSHARD:hi - c * NSHARD])
                                    lo = hi
                            ntile_s = (ncols + P - 1) // P
                            for b0 in range(0, ntile_s, BST):
                                nb = min(BST, ntile_s - b0)
                                rowt = stp.tile([P, BST, WCOLS], FP16, tag="rowt")
                                for q in range(nb):
                                    k = b0 + q
                                    rows = min(P, ncols - k * P)
                                    hp = ps.tile([P, WCOLS], F32, tag="hp")
                                    nc.tensor.matmul(hp[:rows, :],
                                                     lhsT=slab[:, k * P:k * P + rows],
                                                     rhs=wext[:],
                                                     start=True, stop=True)
                                    # GPSIMD cannot read PSUM: rotate DVE/Act
                                    eng = (nc.vector, nc.scalar)[cast_i % 2]
                                    cast_i += 1
                                    if eng is nc.scalar:
                                        nc.scalar.activation(rowt[:rows, q, :],
                                                             hp[:rows, :], ACTF.Copy)
                                    else:
                                        nc.vector.tensor_copy(out=rowt[:rows, q, :],
                                                              in_=hp[:rows, :])
                                rows_b = min(BST * P, ncols - b0 * P)
                                nfull = rows_b // P
                                dst_r0 = r0 + b0 * P
                                if nfull:
                                    trows = table[dst_r0:dst_r0 + nfull * P, 0:WCOLS]
                                    nc.sync.dma_start(
                                        out=trows.rearrange("(q p) c -> p q c", p=P),
                                        in_=rowt[:, 0:nfull, :])
                                tail = rows_b - nfull * P
                                if tail:
                                    nc.sync.dma_start(
                                        out=table[dst_r0 + nfull * P:
                                                  dst_r0 + rows_b, 0:WCOLS],
                                        in_=rowt[:tail, nfull, :])

                # --------------- own-er prologue (per-chunk er) --------------
                def er_prologue(xt_src, wext, dest):
                    with tc.tile_pool(name="er_sb", bufs=2) as sb, \
                         tc.tile_pool(name="er_ps", bufs=1, space="PSUM") as ps:
                        erp_all = ps.tile([P, NCHUNK, 4], F32, tag="erp_all")
                        for j in range(NCHUNK):
                            rows = min(P, NSHARD - j * P)
                            nc.tensor.matmul(erp_all[:rows, j, :],
                                             lhsT=xt_src[:, j * P:j * P + rows],
                                             rhs=wext[:, WCOLS - 4:WCOLS],
                                             start=True, stop=True)
                        nfull = NCHUNK - 1
                        nc.vector.tensor_copy(out=dest[:, 0:nfull, :],
                                              in_=erp_all[:, 0:nfull, :])
                        lrows = NSHARD - (NCHUNK - 1) * P
                        nc.vector.tensor_copy(out=dest[:lrows, nfull, :],
                                              in_=erp_all[:lrows, nfull, :])

                # ------------------------- edge phase ------------------------
                def edge_phase(table, ero, bias_c, layer):
                    nch = NCHUNK if max_chunks is None else min(max_chunks, NCHUNK)
                    with tc.tile_pool(name="eg", bufs=3) as eg, \
                         tc.tile_pool(name="em", bufs=2) as em, \
                         tc.tile_pool(name="emt", bufs=2) as emt, \
                         tc.tile_pool(name="es", bufs=3) as es_pool, \
                         tc.tile_pool(name="eps", bufs=1, space="PSUM") as eps, \
                         tc.tile_pool(name="epa", bufs=2, space="PSUM") as epa, \
                         tc.tile_pool(name="epe", bufs=2, space="PSUM") as epe, \
                         tc.tile_pool(name="epx", bufs=1, space="PSUM") as epx:
                        toff = 0
                        for j in range(nch):
                            Tl, Th = T_lo[j], T_hi[j]
                            T = Tl + Th
                            rows = min(P, NSHARD - j * P)
                            gbuf = eg.tile([P, TMAX, ROW], FP16, tag="gbuf")
                            nc.gpsimd.dma_gather(
                                out_ap=gbuf[:, 0:Tl, :], in_ap=table[0:SPLIT, :],
                                idxs_ap=idx_sb[:, 8 * toff:8 * (toff + Tl)],
                                num_idxs=P * Tl, num_idxs_reg=P * Tl,
                                elem_size=ROW, single_packet=False)
                            if Th:
                                nc.gpsimd.dma_gather(
                                    out_ap=gbuf[:, Tl:T, :], in_ap=table[SPLIT:N, :],
                                    idxs_ap=idx_sb[:, 8 * (toff + Tl):8 * (toff + T)],
                                    num_idxs=P * Th, num_idxs_reg=P * Th,
                                    elem_size=ROW, single_packet=False)
                            # dst mask m_all[p, c, t] = (c == dlc[p, t])
                            m_all = em.tile([P, P, TMAX], FP16, tag="m_all")
                            dsl = dlc_sb[:, toff:toff + T]
                            dlc_b = bass.AP(tensor=dsl.tensor, offset=dsl.offset,
                                            ap=[dsl.ap[0], [0, P]] + dsl.ap[1:])
                            nc.vector.tensor_tensor(out=m_all[:, :, 0:T],
                                                    in0=iotaw[:, :, 0:T],
                                                    in1=dlc_b, op=ALU.is_equal)
                            # mt = transpose(m_all) per tile, via PE + one copy
                            mtp = eps.tile([P, TMAX, P], FP16, tag="mtp")
                            for t in range(T):
                                nc.tensor.transpose(out=mtp[:, t, :],
                                                    in_=m_all[:, :, t],
                                                    identity=identb[:])
                            mt = emt.tile([P, TMAX, P], FP16, tag="mt")
                            nc.scalar.activation(mt[:, 0:T, :], mtp[:, 0:T, :],
                                                 ACTF.Copy)
                            # er per slot: erp[p, t, :] = mt_t^T @ ero
                            erp = epe.tile([P, TMAX, 4], F32, tag="erp")
                            for t in range(T):
                                nc.tensor.matmul(erp[:, t, :], lhsT=mt[:, t, :],
                                                 rhs=ero[:, j, :],
                                                 start=True, stop=True)
                            # e = el + er ; lrelu ; exp -> gbuf[...,260:264]
                            e_sb = es_pool.tile([P, TMAX, 4], F32, tag="e_sb")
                            nc.vector.tensor_tensor(out=e_sb[:, 0:T, :],
                                                    in0=gbuf[:, 0:T, ELOFF:ELOFF + 4],
                                                    in1=erp[:, 0:T, :], op=ALU.add)
                            lr = es_pool.tile([P, TMAX, 4], F32, tag="lr")
                            nc.scalar.activation(lr[:, 0:T, :], e_sb[:, 0:T, :],
                                                 ACTF.Prelu, alpha=NEG)
                            gex = gbuf[:, 0, EROFF:EROFF + 4]
                            ex_out = bass.AP(tensor=gex.tensor, offset=gex.offset,
                                             ap=[gex.ap[0], [ROW, T], [1, 4]])
                            nc.scalar.activation(ex_out, lr[:, 0:T, :], ACTF.Exp)
                            # h *= ex (2x-rate thanks to (d,h) interleave)
                            gb0 = gbuf[:, 0, 0:HD]
                            hv = bass.AP(tensor=gb0.tensor, offset=gb0.offset,
                                         ap=[gb0.ap[0], [ROW, T], [H, D], [1, H]])
                            ex_b = bass.AP(tensor=gex.tensor, offset=gex.offset,
                                           ap=[gex.ap[0], [ROW, T], [0, D], [1, H]])
                            nc.vector.tensor_tensor(out=hv, in0=hv, in1=ex_b,
                                                    op=ALU.mult)
                            # aggregate: T accumulating mask matmuls
                            agg = epa.tile([P, WCOLS], F32, tag="agg")
                            for t in range(T):
                                nc.tensor.matmul(agg[:], lhsT=m_all[:, :, t],
                                                 rhs=gbuf[:, t, 0:WCOLS],
                                                 start=(t == 0), stop=(t == T - 1))
                            # node-space epilogue on DVE(recip) + GpSimd
                            sr = es_pool.tile([P, 4], F32, tag="sr")
                            nc.vector.reciprocal(sr[:], agg[:, EROFF:EROFF + 4])
                            agg_r = agg[:, 0:HD].rearrange("p (d h) -> p d h", h=H)
                            sr_ap = sr[:]
                            sr_b = bass.AP(tensor=sr_ap.tensor, offset=sr_ap.offset,
                                           ap=[sr_ap.ap[0], [0, D], [1, H]])
                            osb = es_pool.tile([P, D, H], F32, tag="osb")
                            # agg is PSUM: this one stays on DVE (GPSIMD
                            # cannot read PSUM); the rest go to GpSimd.
                            nc.vector.tensor_tensor(out=osb[:], in0=agg_r,
                                                    in1=sr_b, op=ALU.mult)
                            bias_r = bias_c[:].rearrange("p (d h) -> p d h", h=H)
                            nc.vector.tensor_tensor(out=osb[:], in0=osb[:],
                                                    in1=bias_r, op=ALU.add)
                            # 0.25*relu(x) == relu(0.25*x): head-mean scale
                            # folded into the Act scale.
                            nc.scalar.activation(osb[:], osb[:], ACTF.Relu,
                                                 scale=0.25)
                            xo = es_pool.tile([P, D], F32, tag="xo")
                            nc.vector.tensor_reduce(
                                out=xo[:], in_=osb[:],
                                axis=mybir.AxisListType.X, op=ALU.add)
                            # transpose -> [D, rows] and store into xT slab
                            xop = epx.tile([D, P], F32, tag="xop")
                            nc.tensor.transpose(out=xop[:, 0:rows],
                                                in_=xo[:rows, :],
                                                identity=identf[:rows, :rows])
                            if layer == 1:
                                nc.vector.tensor_copy(
                                    out=x2t_sb[:, j * P:j * P + rows],
                                    in_=xop[:, 0:rows])
                            else:
                                nc.vector.tensor_copy(
                                    out=x3t_sb[:, j * P:j * P + rows],
                                    in_=xop[:, 0:rows])
                            toff += T

                # ------------------------------ go ---------------------------
                order = ["P1", "E1", "AG", "P2", "E2", "full"]
                upto = order.index(phases)
                done = False

                projection(1)
                er_prologue(fownt, w1ext, erown1)
                done = upto <= order.index("P1")
                if not done:
                    edge_phase(table1, erown1, b1q, layer=1)
                    nc.sync.dma_start(out=x2sliceT[:], in_=x2t_sb[:])
                    done = upto <= order.index("E1")
                if not done:
                    nc.gpsimd.collective_compute(
                        "AllGather", mybir.AluOpType.bypass,
                        replica_groups=[list(range(NCORES))],
                        ins=[x2sliceT[:]], outs=[x2fullT[:]])
                    done = upto <= order.index("AG")
                if not done:
                    projection(2)
                    er_prologue(x2t_sb, w2ext, erown2)
                    done = upto <= order.index("P2")
                if not done:
                    edge_phase(table2, erown2, b2q, layer=2)
                    done = upto <= order.index("E2")
                if done:
                    with tc.tile_pool(name="dbg0", bufs=1) as dbg0:
                        z = dbg0.tile([P, NCLS], F32, tag="dbgz")
                        nc.vector.memset(z[:], 0.0)
                        for j in range(NCHUNK):
                            r0 = j * P
                            rows = min(P, NSHARD - r0)
                            nc.sync.dma_start(out=out[r0:r0 + rows, :], in_=z[:rows])
                    raise _SkipRest()

                # ------------------------------ MLP --------------------------
                # pass A: z = relu(x3 @ Wm1 + bm1) -> zstore; raw moment sums
                with tc.tile_pool(name="ma", bufs=3) as ma, \
                     tc.tile_pool(name="map", bufs=2, space="PSUM") as map_, \
                     tc.tile_pool(name="sta", bufs=1, space="PSUM") as sta:
                    sa1 = sta.tile([P, 1], F32, tag="sa1")
                    sa2 = sta.tile([P, 1], F32, tag="sa2")
                    sq1 = sta.tile([P, 1], F32, tag="sq1")
                    sq2 = sta.tile([P, 1], F32, tag="sq2")
                    for j in range(NCHUNK):
                        rows = min(P, NSHARD - j * P)
                        zp = map_.tile([P, MLP_H], F32, tag="zp")
                        nc.tensor.matmul(zp[:rows, :],
                                         lhsT=x3t_sb[:, j * P:j * P + rows],
                                         rhs=wm1f[:], start=True, stop=True)
                        zc = zstore[:, j, 0:MLP_H]
                        nc.vector.tensor_tensor(out=zc[:rows], in0=zp[:rows],
                                                in1=bm1[:rows], op=ALU.add)
                        nc.scalar.activation(zc[:rows], zc[:rows], ACTF.Relu)
                        zq = ma.tile([P, MLP_H], F32, tag="zq")
                        nc.vector.tensor_tensor(out=zq[:rows], in0=zc[:rows],
                                                in1=zc[:rows], op=ALU.mult)
                        first, last = (j == 0), (j == NCHUNK - 1)
                        nc.tensor.matmul(sa1[:], lhsT=zc[:rows, 0:P],
                                         rhs=onesf[:rows], start=first, stop=last)
                        nc.tensor.matmul(sa2[:MLP_H - P], lhsT=zc[:rows, P:MLP_H],
                                         rhs=onesf[:rows], start=first, stop=last)
                        nc.tensor.matmul(sq1[:], lhsT=zq[:rows, 0:P],
                                         rhs=onesf[:rows], start=first, stop=last)
                        nc.tensor.matmul(sq2[:MLP_H - P], lhsT=zq[:rows, P:MLP_H],
                                         rhs=onesf[:rows], start=first, stop=last)
                    pk = ma.tile([P, 4], F32, tag="pk")
                    nc.vector.memset(pk[:], 0.0)
                    nc.vector.tensor_copy(out=pk[:, 0:1], in_=sa1[:])
                    nc.vector.tensor_copy(out=pk[:MLP_H - P, 1:2], in_=sa2[:MLP_H - P])
                    nc.vector.tensor_copy(out=pk[:, 2:3], in_=sq1[:])
                    nc.vector.tensor_copy(out=pk[:MLP_H - P, 3:4], in_=sq2[:MLP_H - P])
                    nc.sync.dma_start(out=ccin[:], in_=pk[:])

                nc.gpsimd.collective_compute(
                    "AllReduce", mybir.AluOpType.add,
                    replica_groups=[list(range(NCORES))],
                    ins=[ccin[:]], outs=[ccout[:]])

                # pass C: BN constants folded into final weights
                with tc.tile_pool(name="m2", bufs=3) as m2, \
                     tc.tile_pool(name="m2p", bufs=2, space="PSUM") as m2p:
                    stg = m2.tile([P, 4], F32, tag="stg")
                    nc.sync.dma_start(out=stg[:], in_=ccout[:])
                    m1t = m2.tile([P, 2], F32, tag="m1t")
                    nc.vector.tensor_scalar(out=m1t[:], in0=stg[:, 0:2],
                                            scalar1=1.0 / N, scalar2=None,
                                            op0=ALU.mult)
                    m2t = m2.tile([P, 2], F32, tag="m2t")
                    nc.vector.tensor_scalar(out=m2t[:], in0=stg[:, 2:4],
                                            scalar1=1.0 / N, scalar2=None,
                                            op0=ALU.mult)
                    var = m2.tile([P, 2], F32, tag="var")
                    nc.vector.tensor_tensor(out=var[:], in0=m1t[:], in1=m1t[:],
                                            op=ALU.mult)
                    nc.vector.tensor_tensor(out=var[:], in0=m2t[:], in1=var[:],
                                            op=ALU.subtract)
                    nc.vector.tensor_scalar(out=var[:], in0=var[:], scalar1=EPS,
                                            scalar2=None, op0=ALU.add)
                    std = m2.tile([P, 2], F32, tag="std")
                    nc.scalar.activation(std[:], var[:], ACTF.Sqrt)
                    rstd = m2.tile([P, 2], F32, tag="rstd")
                    nc.vector.reciprocal(rstd[:], std[:])
                    gp = m2.tile([P, 2], F32, tag="gp")
                    nc.vector.tensor_tensor(out=gp[:], in0=gb[:, 0:2], in1=rstd[:],
                                            op=ALU.mult)
                    bp = m2.tile([P, 2], F32, tag="bp")
                    nc.vector.tensor_tensor(out=bp[:], in0=m1t[:], in1=gp[:],
                                            op=ALU.mult)
                    nc.vector.tensor_tensor(out=bp[:], in0=gb[:, 2:4], in1=bp[:],
                                            op=ALU.subtract)
                    wp1 = m2.tile([P, NCLS], F32, tag="wp1")
                    nc.vector.tensor_scalar_mul(wp1[:], wm2c1f[:], gp[:, 0:1])
                    wp2 = m2.tile([P, NCLS], F32, tag="wp2")
                    nc.vector.memset(wp2[:], 0.0)
                    nc.vector.tensor_scalar_mul(wp2[:MLP_H - P, :], wm2c2f[:],
                                                gp[:MLP_H - P, 1:2])
                    cp = m2p.tile([1, NCLS], F32, tag="cp")
                    nc.tensor.matmul(cp[:], lhsT=bp[:, 0:1], rhs=wm2c1f[:],
                                     start=True, stop=False)
                    nc.tensor.matmul(cp[:], lhsT=bp[:MLP_H - P, 1:2], rhs=wm2c2f[:],
                                     start=False, stop=True)
                    cps = m2.tile([1, NCLS], F32, tag="cps")
                    nc.vector.tensor_tensor(out=cps[:], in0=cp[:], in1=bm2[:],
                                            op=ALU.add)
                    # place c'' into wp2 row (MLP_H - P) — pairs with the
                    # all-ones zstore column MLP_H. DMA: cross-partition move.
                    nc.sync.dma_start(out=wp2[MLP_H - P:MLP_H - P + 1, :],
                                      in_=cps[:])

                    # pass D: out = z @ W'' (+ c'' via ones column)
                    for j in range(NCHUNK):
                        r0 = j * P
                        rows = min(P, NSHARD - r0)
                        t1 = m2p.tile([P, P], F32, tag="t1")
                        nc.tensor.transpose(out=t1[:], in_=zstore[:, j, 0:P],
                                            identity=identf[:])
                        zT1 = m2.tile([P, P], F32, tag="zT1")
                        nc.vector.tensor_copy(out=zT1[:], in_=t1[:])
                        t2 = m2p.tile([MLP_H + 1 - P, P], F32, tag="t2")
                        nc.tensor.transpose(out=t2[:], in_=zstore[:, j, P:MLP_H + 1],
                                            identity=identf[:])
                        zT2 = m2.tile([MLP_H + 1 - P, P], F32, tag="zT2")
                        nc.vector.tensor_copy(out=zT2[:], in_=t2[:])
                        op_ = m2p.tile([P, NCLS], F32, tag="op")
                        nc.tensor.matmul(op_[:], lhsT=zT1[:], rhs=wp1[:],
                                         start=True, stop=False)
                        nc.tensor.matmul(op_[:], lhsT=zT2[:], rhs=wp2[:MLP_H + 1 - P, :],
                                         start=False, stop=True)
                        ofin = m2.tile([P, NCLS], F32, tag="ofin")
                        nc.vector.tensor_copy(out=ofin[:rows], in_=op_[:rows])
                        nc.sync.dma_start(out=out[r0:r0 + rows, :], in_=ofin[:rows])

            for _rep in range(reps):
                _run_once()

        except _SkipRest:
            pass
    nc.finalize()
    return nc


# ----------------------------------------------------------------------------
# Host entry
# ----------------------------------------------------------------------------

def prep_inputs(inputs):
    f32 = np.float32
    perm = _perm()
    W1ext = _fold_weights(np.asarray(inputs["W1"], f32),
                          np.asarray(inputs["al1"], f32),
                          np.asarray(inputs["ar1"], f32))
    W2ext = _fold_weights(np.asarray(inputs["W2"], f32),
                          np.asarray(inputs["al2"], f32),
                          np.asarray(inputs["ar2"], f32))
    plan, earrays = _prep_edges(inputs["src"], inputs["dst"])
    TMAX = plan["TMAX"]

    featT = np.ascontiguousarray(
        np.asarray(inputs["features"], f32).astype(np.float16).T)  # [128, N]
    iotaw = np.broadcast_to(
        np.repeat(np.arange(P, dtype=np.float16), TMAX)[None, :],
        (P, P * TMAX)).copy()
    bc = lambda v: np.broadcast_to(np.asarray(v, f32).reshape(1, -1),
                                   (P, np.asarray(v).size)).copy()
    consts = {
        "IDENT": np.eye(P, dtype=np.float16),
        "IDENTF": np.eye(P, dtype=f32),
        "IOTAW": iotaw,
        "W1EXT": W1ext, "W2EXT": W2ext,
        "B1Q": bc(np.asarray(inputs["b1"], f32)[perm]),
        "B2Q": bc(np.asarray(inputs["b2"], f32)[perm]),
        "WM1F": np.asarray(inputs["Wm1"], f32),
        "BM1": bc(inputs["bm1"]),
        "WM2C1F": np.asarray(inputs["Wm2"], f32)[0:P, :],
        "WM2C2F": np.asarray(inputs["Wm2"], f32)[P:MLP_H, :],
        "BM2": np.asarray(inputs["bm2"], f32).reshape(1, NCLS),
        "ONESF": np.ones((P, 1), f32),
    }
    gbarr = np.zeros((P, 4), f32)
    gamma = np.asarray(inputs["gamma"], f32)
    beta = np.asarray(inputs["beta"], f32)
    gbarr[:, 0] = gamma[0:P]
    gbarr[0:MLP_H - P, 1] = gamma[P:MLP_H]
    gbarr[:, 2] = beta[0:P]
    gbarr[0:MLP_H - P, 3] = beta[P:MLP_H]
    consts["GB"] = gbarr

    in_maps = []
    for c in range(NCORES):
        IDXa, DLCa = earrays[c]
        m = dict(consts)
        m["FEATT"] = featT
        m["FOWNT"] = np.ascontiguousarray(featT[:, c * NSHARD:(c + 1) * NSHARD])
        m["IDX"] = IDXa
        m["DLC"] = DLCa
        in_maps.append(m)
    return plan, in_maps


def kernel(**inputs):
    from concourse.bass_utils import run_bass_kernel_spmd
    plan, in_maps = prep_inputs(inputs)
    nc = build_nc(plan)
    res = run_bass_kernel_spmd(nc, in_maps, core_ids=list(range(NCORES)))
    out = np.concatenate([res.results[c]["out"] for c in range(NCORES)], axis=0)
    return out.astype(np.float32)


if __name__ == "__main__":
    import time
    t0 = time.time()
    plan, _ = _prep_edges(
        np.concatenate([np.random.randint(0, N, 800000), np.arange(N)]).astype(np.int32),
        np.concatenate([np.random.randint(0, N, 800000), np.arange(N)]).astype(np.int32))
    print("edge prep:", time.time() - t0, "totT:", plan["totT"], "TMAX:", plan["TMAX"])
    t0 = time.time()
    nc = build_nc(plan)
    print("build:", time.time() - t0)


# revision 3
# speedup vs baseline: 1.8877x; 1.3597x over previous
"""GAT (2-layer, 4-head) + MLP/BatchNorm predictor on 8 Trainium2 NeuronCores.

v2 strategy (graph-parallel, dst-sharded; engine-balanced):
  - Nodes split contiguously: core c owns dsts [c*6250, (c+1)*6250). Edges live
    with their dst core, sorted by dst, grouped into 49 chunks of <=128 dsts.
  - Features are uploaded TRANSPOSED (featT [128, N] fp16) so the projection
    needs no on-chip transposes: per 128-row tile one fp16 matmul
    x @ [W | Wel | Wer] -> PSUM f32, cast f32->fp16 round-robin on
    DVE/Act/Pool, batched 4-tile DMA into the table (row = 256 h fp16
    (d,h)-interleaved | el 4xfp16 | er 4xfp16 | pad to 768B).
  - Edge phase per chunk: dma_gather of 768B src rows; dst-mask m_all built by
    one DVE is_equal (all-fp16 packed APs -> 2x mode); transpose-mask mt via
    PE transposes of m_all + one DVE copy; er broadcast dst->slot via tiny
    mt matmuls; e = el+er, LeakyReLU on DVE, exp on Act (only Act func in the
    phase -> no activation-table reloads) written straight into the row pad;
    h *= ex as ONE DVE multiply at 2x rate thanks to the (d,h) interleave;
    aggregation + softmax denominators via T accumulating mask matmuls on PE;
    node-space epilogue (1/s scaling, bias, ReLU, head-mean) on GpSimd
    scalar_tensor_tensor ops; output transposed on PE into SBUF-resident
    x2T/x3T slabs.
  - Collectives: AllGather of x2T (fp16, [64, 6250] per core) so each core
    projects the full layer-2 table; single AllReduce of BatchNorm raw
    moment sums (single-pass E[z], E[z^2] in f32).
"""
import sys

sys.path.insert(0, "/opt/trn_rl_repo")

import numpy as np

N = 50000
F_IN = 128
H = 4
D = 64
HD = 256
NCORES = 8
NSHARD = N // NCORES          # 6250
P = 128
NCHUNK = (NSHARD + P - 1) // P  # 49 (last chunk 106 dsts)
SPLIT = 32768                 # int16 gather index limit
MLP_H = 200
NCLS = 2
NEG = 0.2
EPS = 1e-5
ROW = 384                     # fp16 slots per table row (768 B)
ELOFF = 256                   # el at slots 256:260 (fp16), er at 260:264
EROFF = 260


def configure(n, split=32768):
    global N, NSHARD, NCHUNK, SPLIT
    N = n
    NSHARD = N // NCORES
    NCHUNK = (NSHARD + P - 1) // P
    SPLIT = split


# ----------------------------------------------------------------------------
# Host-side preprocessing
# ----------------------------------------------------------------------------

_PERM = None


def _perm():
    """Column permutation (h,d) -> (d,h): new[d*H+h] = old[h*D+d]."""
    global _PERM
    if _PERM is None:
        idx = np.arange(HD).reshape(H, D)          # old[h, d]
        _PERM = idx.T.reshape(-1)                  # new[(d, h)]
    return _PERM


def _fold_weights(W, al, ar):
    """W:[F,H*D] al,ar:[H,D] -> Wext [F, HD+8] f16 with (d,h) interleave."""
    F = W.shape[0]
    W64 = W.astype(np.float64)
    Wel = (W64.reshape(F, H, D) * al[None].astype(np.float64)).sum(-1)  # [F,H]
    Wer = (W64.reshape(F, H, D) * ar[None].astype(np.float64)).sum(-1)
    Wr = W64[:, _perm()]
    return np.concatenate([Wr, Wel, Wer], axis=1).astype(np.float16)


def _prep_edges(src, dst):
    """Per-core gather arrays. Returns (plan, per_core arrays).

    plan: T_lo[j], T_hi[j], totT, TMAX (identical across cores).
    per-core: IDX [128, 8*totT] i16, DLC [128, totT] f16 (pad -1).
    """
    src = np.asarray(src)
    dst = np.asarray(dst)
    per_core = []
    for c in range(NCORES):
        m = (dst >= c * NSHARD) & (dst < (c + 1) * NSHARD)
        es, ed = src[m], dst[m] - c * NSHARD
        order = np.argsort(ed, kind="stable")
        es, ed = es[order], ed[order]
        starts = np.searchsorted(ed, np.arange(0, NCHUNK * P, P))
        ends = np.searchsorted(ed, np.minimum(np.arange(P, (NCHUNK + 1) * P, P), NSHARD))
        chunks = []
        for j in range(NCHUNK):
            cs, ce = starts[j], ends[j]
            s_j, d_j = es[cs:ce], ed[cs:ce] - j * P
            lo = s_j < SPLIT
            chunks.append((s_j[lo], d_j[lo], s_j[~lo] - SPLIT, d_j[~lo]))
        per_core.append(chunks)

    T_lo = np.zeros(NCHUNK, np.int64)
    T_hi = np.zeros(NCHUNK, np.int64)
    for c in range(NCORES):
        for j in range(NCHUNK):
            slo, _, shi, _ = per_core[c][j]
            T_lo[j] = max(T_lo[j], -(-len(slo) // P))
            T_hi[j] = max(T_hi[j], -(-len(shi) // P))
    T_lo = np.maximum(T_lo, 1)
    totT = int((T_lo + T_hi).sum())
    TMAX = int((T_lo + T_hi).max())

    def wrap_idx(flat):
        n = len(flat)
        cols = n // 16
        a = flat.reshape(cols, 16).T.astype(np.int16)      # [16, cols]
        return np.tile(a, (8, 1))                          # [128, cols]

    arrays = []
    for c in range(NCORES):
        idx_cols = []
        dlc = np.full((P, totT), -1.0, np.float16)
        t0 = 0
        for j in range(NCHUNK):
            slo, dlo, shi, dhi = per_core[c][j]
            for (s_j, d_j, T) in ((slo, dlo, T_lo[j]), (shi, dhi, T_hi[j])):
                nslot = int(T) * P
                if nslot == 0:
                    continue
                idx = np.zeros(nslot, np.int16)
                dl = np.full(nslot, -1.0, np.float32)
                idx[: len(s_j)] = s_j
                dl[: len(s_j)] = d_j
                idx_cols.append(wrap_idx(idx))
                dlc[:, t0 : t0 + int(T)] = dl.reshape(int(T), P).T.astype(np.float16)
                t0 += int(T)
        assert t0 == totT
        IDX = np.concatenate(idx_cols, axis=1)
        assert IDX.shape == (P, 8 * totT)
        arrays.append((IDX, dlc))

    plan = {"T_lo": T_lo.tolist(), "T_hi": T_hi.tolist(), "totT": totT,
            "TMAX": TMAX}
    return plan, arrays


# ----------------------------------------------------------------------------
# Bass program
# ----------------------------------------------------------------------------

def build_nc(plan, phases='full', reps=1, max_chunks=None):
    import concourse.bacc as bacc
    import concourse.bass as bass
    import concourse.tile as tile
    from concourse import mybir

    FP16 = mybir.dt.float16
    F32 = mybir.dt.float32
    I16 = mybir.dt.int16
    ALU = mybir.AluOpType
    ACTF = mybir.ActivationFunctionType

    T_lo, T_hi, totT = plan["T_lo"], plan["T_hi"], plan["totT"]
    TMAX = plan["TMAX"]
    NTILE = (N + P - 1) // P            # 391 (last 80 rows)
    WCOLS = HD + 8

    nc = bacc.Bacc("TRN2", target_bir_lowering=False, debug=False,
                   num_devices=NCORES)

    dp = lambda name, shape, dt: nc.declare_dram_parameter(name, shape, dt, isOutput=False)
    FEATT = dp("FEATT", [P, N], FP16)
    FOWNT = dp("FOWNT", [P, NSHARD], FP16)
    IDX = dp("IDX", [P, 8 * totT], I16)
    DLC = dp("DLC", [P, totT], FP16)
    IOTAW = dp("IOTAW", [P, P * TMAX], FP16)
    IDENT = dp("IDENT", [P, P], FP16)
    IDENTF = dp("IDENTF", [P, P], F32)
    W1EXT = dp("W1EXT", [F_IN, WCOLS], FP16)
    W2EXT = dp("W2EXT", [D, WCOLS], FP16)
    B1Q = dp("B1Q", [P, HD], F32)
    B2Q = dp("B2Q", [P, HD], F32)
    WM1F = dp("WM1F", [D, MLP_H], F32)
    BM1 = dp("BM1", [P, MLP_H], F32)
    WM2C1F = dp("WM2C1F", [P, NCLS], F32)
    WM2C2F = dp("WM2C2F", [MLP_H - P, NCLS], F32)
    GB = dp("GB", [P, 4], F32)
    BM2 = dp("BM2", [1, NCLS], F32)
    ONESF = dp("ONESF", [P, 1], F32)

    out = nc.declare_dram_parameter("out", [NSHARD, NCLS], F32, isOutput=True)

    table1 = nc.dram_tensor("table1", [N, ROW], FP16)
    table2 = nc.dram_tensor("table2", [N, ROW], FP16)
    AGSPL = 25 * P                       # AllGather stage split (3200)
    x2sliceA = nc.dram_tensor("x2sliceA", [D, AGSPL], FP16)
    x2sliceB = nc.dram_tensor("x2sliceB", [D, NSHARD - AGSPL], FP16)
    x2fullA = nc.dram_tensor("x2fullA", [NCORES * D, AGSPL], FP16,
                             addr_space="Shared")
    x2fullB = nc.dram_tensor("x2fullB", [NCORES * D, NSHARD - AGSPL], FP16,
                             addr_space="Shared")
    ccin = nc.dram_tensor("ccin", [P, 4], F32)
    ccout = nc.dram_tensor("ccout", [P, 4], F32, addr_space="Shared")

    class _SkipRest(Exception):
        pass

    with tile.TileContext(nc) as tc:
        import contextlib
        try:
          with contextlib.ExitStack() as ctx:
            singles = ctx.enter_context(tc.tile_pool(name="singles", bufs=1))

            def load_const(param, shape, dtype, tag):
                t = singles.tile(shape, dtype, tag=tag)
                nc.sync.dma_start(out=t[:], in_=param[:])
                return t

            identb = load_const(IDENT, [P, P], FP16, "c_ident")
            identf = load_const(IDENTF, [P, P], F32, "c_identf")
            iotaw = load_const(IOTAW, [P, P, TMAX], FP16, "c_iotaw")
            w1ext = load_const(W1EXT, [F_IN, WCOLS], FP16, "c_w1ext")
            w2ext = load_const(W2EXT, [D, WCOLS], FP16, "c_w2ext")
            b1q = load_const(B1Q, [P, HD], F32, "c_b1q")
            b2q = load_const(B2Q, [P, HD], F32, "c_b2q")
            wm1f = load_const(WM1F, [D, MLP_H], F32, "c_wm1f")
            bm1 = load_const(BM1, [P, MLP_H], F32, "c_bm1")
            wm2c1f = load_const(WM2C1F, [P, NCLS], F32, "c_wm2c1f")
            wm2c2f = load_const(WM2C2F, [MLP_H - P, NCLS], F32, "c_wm2c2f")
            gb = load_const(GB, [P, 4], F32, "c_gb")
            bm2 = load_const(BM2, [1, NCLS], F32, "c_bm2")
            onesf = load_const(ONESF, [P, 1], F32, "c_onesf")
            fownt = load_const(FOWNT, [P, NSHARD], FP16, "c_fownt")
            idx_sb = load_const(IDX, [P, 8 * totT], I16, "c_idx")
            dlc_sb = load_const(DLC, [P, totT], FP16, "c_dlc")

            x2t_sb = singles.tile([D, NSHARD], FP16, tag="c_x2t")
            x3t_sb = singles.tile([D, NSHARD], F32, tag="c_x3t")
            erown1 = singles.tile([P, NCHUNK, 4], FP16, tag="c_erown1")
            erown2 = singles.tile([P, NCHUNK, 4], FP16, tag="c_erown2")
            zstore = singles.tile([P, NCHUNK, MLP_H + 1], F32, tag="c_zstore")

            def _run_once():
                nc.vector.memset(erown1[:], 0.0)
                nc.vector.memset(erown2[:], 0.0)
                nc.vector.memset(zstore[:], 0.0)
                # ones column (col MLP_H) for the pass-D folded-constant row
                nc.vector.memset(zstore[:, :, MLP_H:MLP_H + 1], 1.0)

                # ---------------- projection phase (full table) --------------
                def projection(layer):
                    """layer 1: featT -> table1; layer 2: x2fullT -> table2."""
                    F = F_IN if layer == 1 else D
                    wext = w1ext if layer == 1 else w2ext
                    table = table1 if layer == 1 else table2
                    SLAB = 8                     # tiles per load slab
                    BST = 8                      # tiles per store batch
                    with tc.tile_pool(name="proj_sb", bufs=3) as sb, \
                         tc.tile_pool(name="proj_st", bufs=2) as stp, \
                         tc.tile_pool(name="proj_ps", bufs=4, space="PSUM") as ps:
                        nslab = (NTILE + SLAB - 1) // SLAB
                        cast_i = 0
                        for s in range(nslab):
                            r0 = s * SLAB * P
                            ncols = min(SLAB * P, N - r0)
                            slab = sb.tile([F, SLAB * P], FP16, tag="slab")
                            if layer == 1:
                                nc.sync.dma_start(out=slab[:, 0:ncols],
                                                  in_=FEATT[:, r0:r0 + ncols])
                            else:
                                # x2full rows live in per-core 64-row bands,
                                # split at AGSPL into the A/B staged tensors
                                lo = r0
                                while lo < r0 + ncols:
                                    c = lo // NSHARD
                                    lc = lo - c * NSHARD
                                    if lc < AGSPL:
                                        hi = min(r0 + ncols,
                                                 c * NSHARD + AGSPL)
                                        srct, off = x2fullA, lc
                                    else:
                                        hi = min(r0 + ncols, (c + 1) * NSHARD)
                                        srct, off = x2fullB, lc - AGSPL
                                    nc.sync.dma_start(
                                        out=slab[:, lo - r0:hi - r0],
                                        in_=srct[c * D:(c + 1) * D,
                                                 off:off + (hi - lo)])
                                    lo = hi
                            ntile_s = (ncols + P - 1) // P
                            for b0 in range(0, ntile_s, BST):
                                nb = min(BST, ntile_s - b0)
                                rowt = stp.tile([P, BST, WCOLS], FP16, tag="rowt")
                                for q in range(nb):
                                    k = b0 + q
                                    rows = min(P, ncols - k * P)
                                    hp = ps.tile([P, WCOLS], F32, tag="hp")
                                    nc.tensor.matmul(hp[:rows, :],
                                                     lhsT=slab[:, k * P:k * P + rows],
                                                     rhs=wext[:],
                                                     start=True, stop=True)
                                    # GPSIMD cannot read PSUM: rotate
                                    # 1:2 DVE:Act (DVE is the busier engine)
                                    eng = (nc.vector, nc.scalar,
                                           nc.scalar)[cast_i % 3]
                                    cast_i += 1
                                    if eng is nc.scalar:
                                        nc.scalar.activation(rowt[:rows, q, :],
                                                             hp[:rows, :], ACTF.Copy)
                                    else:
                                        nc.vector.tensor_copy(out=rowt[:rows, q, :],
                                                              in_=hp[:rows, :])
                                rows_b = min(BST * P, ncols - b0 * P)
                                nfull = rows_b // P
                                dst_r0 = r0 + b0 * P
                                if nfull:
                                    trows = table[dst_r0:dst_r0 + nfull * P, 0:WCOLS]
                                    nc.sync.dma_start(
                                        out=trows.rearrange("(q p) c -> p q c", p=P),
                                        in_=rowt[:, 0:nfull, :])
                                tail = rows_b - nfull * P
                                if tail:
                                    nc.sync.dma_start(
                                        out=table[dst_r0 + nfull * P:
                                                  dst_r0 + rows_b, 0:WCOLS],
                                        in_=rowt[:tail, nfull, :])

                # --------------- own-er prologue (per-chunk er) --------------
                def er_prologue(xt_src, wext, dest):
                    with tc.tile_pool(name="er_sb", bufs=2) as sb, \
                         tc.tile_pool(name="er_ps", bufs=1, space="PSUM") as ps:
                        erp_all = ps.tile([P, NCHUNK, 4], F32, tag="erp_all")
                        for j in range(NCHUNK):
                            rows = min(P, NSHARD - j * P)
                            nc.tensor.matmul(erp_all[:rows, j, :],
                                             lhsT=xt_src[:, j * P:j * P + rows],
                                             rhs=wext[:, WCOLS - 4:WCOLS],
                                             start=True, stop=True)
                        nfull = NCHUNK - 1
                        nc.vector.tensor_copy(out=dest[:, 0:nfull, :],
                                              in_=erp_all[:, 0:nfull, :])
                        lrows = NSHARD - (NCHUNK - 1) * P
                        nc.vector.tensor_copy(out=dest[:lrows, nfull, :],
                                              in_=erp_all[:lrows, nfull, :])

                # ------------------------- edge phase ------------------------
                def edge_phase(table, ero, bias_c, layer):
                    nch = NCHUNK if max_chunks is None else min(max_chunks, NCHUNK)
                    with tc.tile_pool(name="eg", bufs=3) as eg, \
                         tc.tile_pool(name="em", bufs=2) as em, \
                         tc.tile_pool(name="emt", bufs=2) as emt, \
                         tc.tile_pool(name="es", bufs=3) as es_pool, \
                         tc.tile_pool(name="eps", bufs=1, space="PSUM") as eps, \
                         tc.tile_pool(name="epa", bufs=2, space="PSUM") as epa, \
                         tc.tile_pool(name="epe", bufs=2, space="PSUM") as epe, \
                         tc.tile_pool(name="epx", bufs=1, space="PSUM") as epx:
                        toff = 0
                        for j in range(nch):
                            Tl, Th = T_lo[j], T_hi[j]
                            T = Tl + Th
                            rows = min(P, NSHARD - j * P)
                            gbuf = eg.tile([P, TMAX, ROW], FP16, tag="gbuf")
                            nc.gpsimd.dma_gather(
                                out_ap=gbuf[:, 0:Tl, :], in_ap=table[0:SPLIT, :],
                                idxs_ap=idx_sb[:, 8 * toff:8 * (toff + Tl)],
                                num_idxs=P * Tl, num_idxs_reg=P * Tl,
                                elem_size=ROW, single_packet=False)
                            if Th:
                                nc.gpsimd.dma_gather(
                                    out_ap=gbuf[:, Tl:T, :], in_ap=table[SPLIT:N, :],
                                    idxs_ap=idx_sb[:, 8 * (toff + Tl):8 * (toff + T)],
                                    num_idxs=P * Th, num_idxs_reg=P * Th,
                                    elem_size=ROW, single_packet=False)
                            # dst mask m_all[p, c, t] = (c == dlc[p, t])
                            m_all = em.tile([P, P, TMAX], FP16, tag="m_all")
                            dsl = dlc_sb[:, toff:toff + T]
                            dlc_b = bass.AP(tensor=dsl.tensor, offset=dsl.offset,
                                            ap=[dsl.ap[0], [0, P]] + dsl.ap[1:])
                            nc.vector.tensor_tensor(out=m_all[:, :, 0:T],
                                                    in0=iotaw[:, :, 0:T],
                                                    in1=dlc_b, op=ALU.is_equal)
                            # mt = transpose(m_all) per tile, via PE + one copy
                            mtp = eps.tile([P, TMAX, P], FP16, tag="mtp")
                            for t in range(T):
                                nc.tensor.transpose(out=mtp[:, t, :],
                                                    in_=m_all[:, :, t],
                                                    identity=identb[:])
                            mt = emt.tile([P, TMAX, P], FP16, tag="mt")
                            nc.scalar.activation(mt[:, 0:T, :], mtp[:, 0:T, :],
                                                 ACTF.Copy)
                            # er per slot: erp[p, t, :] = mt_t^T @ ero
                            erp = epe.tile([P, TMAX, 4], F32, tag="erp")
                            for t in range(T):
                                nc.tensor.matmul(erp[:, t, :], lhsT=mt[:, t, :],
                                                 rhs=ero[:, j, :],
                                                 start=True, stop=True)
                            # e = el + er ; lrelu ; exp -> gbuf[...,260:264]
                            e_sb = es_pool.tile([P, TMAX, 4], F32, tag="e_sb")
                            nc.vector.tensor_tensor(out=e_sb[:, 0:T, :],
                                                    in0=gbuf[:, 0:T, ELOFF:ELOFF + 4],
                                                    in1=erp[:, 0:T, :], op=ALU.add)
                            lr = es_pool.tile([P, TMAX, 4], F32, tag="lr")
                            nc.scalar.activation(lr[:, 0:T, :], e_sb[:, 0:T, :],
                                                 ACTF.Prelu, alpha=NEG)
                            gex = gbuf[:, 0, EROFF:EROFF + 4]
                            ex_out = bass.AP(tensor=gex.tensor, offset=gex.offset,
                                             ap=[gex.ap[0], [ROW, T], [1, 4]])
                            nc.scalar.activation(ex_out, lr[:, 0:T, :], ACTF.Exp)
                            # h *= ex (2x-rate thanks to (d,h) interleave)
                            gb0 = gbuf[:, 0, 0:HD]
                            hv = bass.AP(tensor=gb0.tensor, offset=gb0.offset,
                                         ap=[gb0.ap[0], [ROW, T], [H, D], [1, H]])
                            ex_b = bass.AP(tensor=gex.tensor, offset=gex.offset,
                                           ap=[gex.ap[0], [ROW, T], [0, D], [1, H]])
                            nc.vector.tensor_tensor(out=hv, in0=hv, in1=ex_b,
                                                    op=ALU.mult)
                            # aggregate: T accumulating mask matmuls
                            agg = epa.tile([P, WCOLS], F32, tag="agg")
                            for t in range(T):
                                nc.tensor.matmul(agg[:], lhsT=m_all[:, :, t],
                                                 rhs=gbuf[:, t, 0:WCOLS],
                                                 start=(t == 0), stop=(t == T - 1))
                            # node-space epilogue on DVE(recip) + GpSimd
                            sr = es_pool.tile([P, 4], F32, tag="sr")
                            nc.vector.reciprocal(sr[:], agg[:, EROFF:EROFF + 4])
                            agg_r = agg[:, 0:HD].rearrange("p (d h) -> p d h", h=H)
                            sr_ap = sr[:]
                            sr_b = bass.AP(tensor=sr_ap.tensor, offset=sr_ap.offset,
                                           ap=[sr_ap.ap[0], [0, D], [1, H]])
                            osb = es_pool.tile([P, D, H], F32, tag="osb")
                            # agg is PSUM: this one stays on DVE (GPSIMD
                            # cannot read PSUM); the rest go to GpSimd.
                            nc.vector.tensor_tensor(out=osb[:], in0=agg_r,
                                                    in1=sr_b, op=ALU.mult)
                            bias_r = bias_c[:].rearrange("p (d h) -> p d h", h=H)
                            nc.vector.tensor_tensor(out=osb[:], in0=osb[:],
                                                    in1=bias_r, op=ALU.add)
                            # 0.25*relu(x) == relu(0.25*x): head-mean scale
                            # folded into the Act scale.
                            nc.scalar.activation(osb[:], osb[:], ACTF.Relu,
                                                 scale=0.25)
                            xo = es_pool.tile([P, D], F32, tag="xo")
                            nc.vector.tensor_reduce(
                                out=xo[:], in_=osb[:],
                                axis=mybir.AxisListType.X, op=ALU.add)
                            # transpose -> [D, rows] and store into xT slab
                            xop = epx.tile([D, P], F32, tag="xop")
                            nc.tensor.transpose(out=xop[:, 0:rows],
                                                in_=xo[:rows, :],
                                                identity=identf[:rows, :rows])
                            if layer == 1:
                                nc.scalar.activation(
                                    x2t_sb[:, j * P:j * P + rows],
                                    xop[:, 0:rows], ACTF.Copy)
                            else:
                                nc.scalar.activation(
                                    x3t_sb[:, j * P:j * P + rows],
                                    xop[:, 0:rows], ACTF.Copy)
                            if layer == 1 and j == 24:
                                nc.sync.dma_start(out=x2sliceA[:],
                                                  in_=x2t_sb[:, 0:AGSPL])
                                nc.gpsimd.collective_compute(
                                    "AllGather", mybir.AluOpType.bypass,
                                    replica_groups=[list(range(NCORES))],
                                    ins=[x2sliceA[:]], outs=[x2fullA[:]])
                            toff += T

                # ------------------------------ go ---------------------------
                order = ["P1", "E1", "AG", "P2", "E2", "full"]
                upto = order.index(phases)
                done = False

                projection(1)
                er_prologue(fownt, w1ext, erown1)
                done = upto <= order.index("P1")
                if not done:
                    edge_phase(table1, erown1, b1q, layer=1)
                    nc.sync.dma_start(out=x2sliceB[:],
                                      in_=x2t_sb[:, AGSPL:NSHARD])
                    done = upto <= order.index("E1")
                if not done:
                    nc.gpsimd.collective_compute(
                        "AllGather", mybir.AluOpType.bypass,
                        replica_groups=[list(range(NCORES))],
                        ins=[x2sliceB[:]], outs=[x2fullB[:]])
                    done = upto <= order.index("AG")
                if not done:
                    projection(2)
                    er_prologue(x2t_sb, w2ext, erown2)
                    done = upto <= order.index("P2")
                if not done:
                    edge_phase(table2, erown2, b2q, layer=2)
                    done = upto <= order.index("E2")
                if done:
                    with tc.tile_pool(name="dbg0", bufs=1) as dbg0:
                        z = dbg0.tile([P, NCLS], F32, tag="dbgz")
                        nc.vector.memset(z[:], 0.0)
                        for j in range(NCHUNK):
                            r0 = j * P
                            rows = min(P, NSHARD - r0)
                            nc.sync.dma_start(out=out[r0:r0 + rows, :], in_=z[:rows])
                    raise _SkipRest()

                # ------------------------------ MLP --------------------------
                # pass A: z = relu(x3 @ Wm1 + bm1) -> zstore; raw moment sums
                with tc.tile_pool(name="ma", bufs=3) as ma, \
                     tc.tile_pool(name="map", bufs=2, space="PSUM") as map_, \
                     tc.tile_pool(name="sta", bufs=1, space="PSUM") as sta:
                    sa1 = sta.tile([P, 1], F32, tag="sa1")
                    sa2 = sta.tile([P, 1], F32, tag="sa2")
                    sq1 = sta.tile([P, 1], F32, tag="sq1")
                    sq2 = sta.tile([P, 1], F32, tag="sq2")
                    for j in range(NCHUNK):
                        rows = min(P, NSHARD - j * P)
                        zp = map_.tile([P, MLP_H], F32, tag="zp")
                        nc.tensor.matmul(zp[:rows, :],
                                         lhsT=x3t_sb[:, j * P:j * P + rows],
                                         rhs=wm1f[:], start=True, stop=True)
                        zc = zstore[:, j, 0:MLP_H]
                        nc.vector.tensor_tensor(out=zc[:rows], in0=zp[:rows],
                                                in1=bm1[:rows], op=ALU.add)
                        nc.scalar.activation(zc[:rows], zc[:rows], ACTF.Relu)
                        zq = ma.tile([P, MLP_H], F32, tag="zq")
                        nc.vector.tensor_tensor(out=zq[:rows], in0=zc[:rows],
                                                in1=zc[:rows], op=ALU.mult)
                        first, last = (j == 0), (j == NCHUNK - 1)
                        nc.tensor.matmul(sa1[:], lhsT=zc[:rows, 0:P],
                                         rhs=onesf[:rows], start=first, stop=last)
                        nc.tensor.matmul(sa2[:MLP_H - P], lhsT=zc[:rows, P:MLP_H],
                                         rhs=onesf[:rows], start=first, stop=last)
                        nc.tensor.matmul(sq1[:], lhsT=zq[:rows, 0:P],
                                         rhs=onesf[:rows], start=first, stop=last)
                        nc.tensor.matmul(sq2[:MLP_H - P], lhsT=zq[:rows, P:MLP_H],
                                         rhs=onesf[:rows], start=first, stop=last)
                    pk = ma.tile([P, 4], F32, tag="pk")
                    nc.vector.memset(pk[:], 0.0)
                    nc.vector.tensor_copy(out=pk[:, 0:1], in_=sa1[:])
                    nc.vector.tensor_copy(out=pk[:MLP_H - P, 1:2], in_=sa2[:MLP_H - P])
                    nc.vector.tensor_copy(out=pk[:, 2:3], in_=sq1[:])
                    nc.vector.tensor_copy(out=pk[:MLP_H - P, 3:4], in_=sq2[:MLP_H - P])
                    nc.sync.dma_start(out=ccin[:], in_=pk[:])

                nc.gpsimd.collective_compute(
                    "AllReduce", mybir.AluOpType.add,
                    replica_groups=[list(range(NCORES))],
                    ins=[ccin[:]], outs=[ccout[:]])

                # pass C: BN constants folded into final weights
                with tc.tile_pool(name="m2", bufs=3) as m2, \
                     tc.tile_pool(name="m2p", bufs=2, space="PSUM") as m2p:
                    stg = m2.tile([P, 4], F32, tag="stg")
                    nc.sync.dma_start(out=stg[:], in_=ccout[:])
                    m1t = m2.tile([P, 2], F32, tag="m1t")
                    nc.vector.tensor_scalar(out=m1t[:], in0=stg[:, 0:2],
                                            scalar1=1.0 / N, scalar2=None,
                                            op0=ALU.mult)
                    m2t = m2.tile([P, 2], F32, tag="m2t")
                    nc.vector.tensor_scalar(out=m2t[:], in0=stg[:, 2:4],
                                            scalar1=1.0 / N, scalar2=None,
                                            op0=ALU.mult)
                    var = m2.tile([P, 2], F32, tag="var")
                    nc.vector.tensor_tensor(out=var[:], in0=m1t[:], in1=m1t[:],
                                            op=ALU.mult)
                    nc.vector.tensor_tensor(out=var[:], in0=m2t[:], in1=var[:],
                                            op=ALU.subtract)
                    nc.vector.tensor_scalar(out=var[:], in0=var[:], scalar1=EPS,
                                            scalar2=None, op0=ALU.add)
                    std = m2.tile([P, 2], F32, tag="std")
                    nc.scalar.activation(std[:], var[:], ACTF.Sqrt)
                    rstd = m2.tile([P, 2], F32, tag="rstd")
                    nc.vector.reciprocal(rstd[:], std[:])
                    gp = m2.tile([P, 2], F32, tag="gp")
                    nc.vector.tensor_tensor(out=gp[:], in0=gb[:, 0:2], in1=rstd[:],
                                            op=ALU.mult)
                    bp = m2.tile([P, 2], F32, tag="bp")
                    nc.vector.tensor_tensor(out=bp[:], in0=m1t[:], in1=gp[:],
                                            op=ALU.mult)
                    nc.vector.tensor_tensor(out=bp[:], in0=gb[:, 2:4], in1=bp[:],
                                            op=ALU.subtract)
                    wp1 = m2.tile([P, NCLS], F32, tag="wp1")
                    nc.vector.tensor_scalar_mul(wp1[:], wm2c1f[:], gp[:, 0:1])
                    wp2 = m2.tile([P, NCLS], F32, tag="wp2")
                    nc.vector.memset(wp2[:], 0.0)
                    nc.vector.tensor_scalar_mul(wp2[:MLP_H - P, :], wm2c2f[:],
                                                gp[:MLP_H - P, 1:2])
                    cp = m2p.tile([1, NCLS], F32, tag="cp")
                    nc.tensor.matmul(cp[:], lhsT=bp[:, 0:1], rhs=wm2c1f[:],
                                     start=True, stop=False)
                    nc.tensor.matmul(cp[:], lhsT=bp[:MLP_H - P, 1:2], rhs=wm2c2f[:],
                                     start=False, stop=True)
                    cps = m2.tile([1, NCLS], F32, tag="cps")
                    nc.vector.tensor_tensor(out=cps[:], in0=cp[:], in1=bm2[:],
                                            op=ALU.add)
                    # place c'' into wp2 row (MLP_H - P) — pairs with the
                    # all-ones zstore column MLP_H. DMA: cross-partition move.
                    nc.sync.dma_start(out=wp2[MLP_H - P:MLP_H - P + 1, :],
                                      in_=cps[:])

                    # pass D: out = z @ W'' (+ c'' via ones column)
                    for j in range(NCHUNK):
                        r0 = j * P
                        rows = min(P, NSHARD - r0)
                        t1 = m2p.tile([P, P], F32, tag="t1")
                        nc.tensor.transpose(out=t1[:], in_=zstore[:, j, 0:P],
                                            identity=identf[:])
                        zT1 = m2.tile([P, P], F32, tag="zT1")
                        nc.scalar.activation(zT1[:], t1[:], ACTF.Copy)
                        t2 = m2p.tile([MLP_H + 1 - P, P], F32, tag="t2")
                        nc.tensor.transpose(out=t2[:], in_=zstore[:, j, P:MLP_H + 1],
                                            identity=identf[:])
                        zT2 = m2.tile([MLP_H + 1 - P, P], F32, tag="zT2")
                        nc.scalar.activation(zT2[:], t2[:], ACTF.Copy)
                        op_ = m2p.tile([P, NCLS], F32, tag="op")
                        nc.tensor.matmul(op_[:], lhsT=zT1[:], rhs=wp1[:],
                                         start=True, stop=False)
                        nc.tensor.matmul(op_[:], lhsT=zT2[:], rhs=wp2[:MLP_H + 1 - P, :],
                                         start=False, stop=True)
                        ofin = m2.tile([P, NCLS], F32, tag="ofin")
                        nc.vector.tensor_copy(out=ofin[:rows], in_=op_[:rows])
                        nc.sync.dma_start(out=out[r0:r0 + rows, :], in_=ofin[:rows])

            for _rep in range(reps):
                _run_once()

        except _SkipRest:
            pass
    nc.finalize()
    return nc


# ----------------------------------------------------------------------------
# Host entry
# ----------------------------------------------------------------------------

def prep_inputs(inputs):
    f32 = np.float32
    perm = _perm()
    W1ext = _fold_weights(np.asarray(inputs["W1"], f32),
                          np.asarray(inputs["al1"], f32),
                          np.asarray(inputs["ar1"], f32))
    W2ext = _fold_weights(np.asarray(inputs["W2"], f32),
                          np.asarray(inputs["al2"], f32),
                          np.asarray(inputs["ar2"], f32))
    plan, earrays = _prep_edges(inputs["src"], inputs["dst"])
    TMAX = plan["TMAX"]

    featT = np.ascontiguousarray(
        np.asarray(inputs["features"], f32).astype(np.float16).T)  # [128, N]
    iotaw = np.broadcast_to(
        np.repeat(np.arange(P, dtype=np.float16), TMAX)[None, :],
        (P, P * TMAX)).copy()
    bc = lambda v: np.broadcast_to(np.asarray(v, f32).reshape(1, -1),
                                   (P, np.asarray(v).size)).copy()
    consts = {
        "IDENT": np.eye(P, dtype=np.float16),
        "IDENTF": np.eye(P, dtype=f32),
        "IOTAW": iotaw,
        "W1EXT": W1ext, "W2EXT": W2ext,
        "B1Q": bc(np.asarray(inputs["b1"], f32)[perm]),
        "B2Q": bc(np.asarray(inputs["b2"], f32)[perm]),
        "WM1F": np.asarray(inputs["Wm1"], f32),
        "BM1": bc(inputs["bm1"]),
        "WM2C1F": np.asarray(inputs["Wm2"], f32)[0:P, :],
        "WM2C2F": np.asarray(inputs["Wm2"], f32)[P:MLP_H, :],
        "BM2": np.asarray(inputs["bm2"], f32).reshape(1, NCLS),
        "ONESF": np.ones((P, 1), f32),
    }
    gbarr = np.zeros((P, 4), f32)
    gamma = np.asarray(inputs["gamma"], f32)
    beta = np.asarray(inputs["beta"], f32)
    gbarr[:, 0] = gamma[0:P]
    gbarr[0:MLP_H - P, 1] = gamma[P:MLP_H]
    gbarr[:, 2] = beta[0:P]
    gbarr[0:MLP_H - P, 3] = beta[P:MLP_H]
    consts["GB"] = gbarr

    in_maps = []
    for c in range(NCORES):
        IDXa, DLCa = earrays[c]
        m = dict(consts)
        m["FEATT"] = featT
        m["FOWNT"] = np.ascontiguousarray(featT[:, c * NSHARD:(c + 1) * NSHARD])
        m["IDX"] = IDXa
        m["DLC"] = DLCa
        in_maps.append(m)
    return plan, in_maps


def kernel(**inputs):
    from concourse.bass_utils import run_bass_kernel_spmd
    plan, in_maps = prep_inputs(inputs)
    nc = build_nc(plan)
    res = run_bass_kernel_spmd(nc, in_maps, core_ids=list(range(NCORES)))
    out = np.concatenate([res.results[c]["out"] for c in range(NCORES)], axis=0)
    return out.astype(np.float32)


if __name__ == "__main__":
    import time
    t0 = time.time()
    plan, _ = _prep_edges(
        np.concatenate([np.random.randint(0, N, 800000), np.arange(N)]).astype(np.int32),
        np.concatenate([np.random.randint(0, N, 800000), np.arange(N)]).astype(np.int32))
    print("edge prep:", time.time() - t0, "totT:", plan["totT"], "TMAX:", plan["TMAX"])
    t0 = time.time()
    nc = build_nc(plan)
    print("build:", time.time() - t0)
